# revision 1
# baseline (speedup 1.0000x reference)
"""Trainium2 Bass kernel for nn_BatchFlipLoss (NCE batch-flip loss + CE loss).

Math reformulation (validated to rel-err ~1e-7 vs the jax reference):

The reference sums BatchCriterion over 36 flip-class pairs (i,j), j>=i.
For pair (i,j) with x = [f_i; f_j] (f_c = features[c::8], L2-normalized,
B=512 rows each), T=0.1, the loss decomposes over ordered halves (a,b)
(rows of f_a, partner f_b). With E_ab = exp(10*G_ab), G_ab = f_a@f_b.T,
S_ab = rowsum(E_ab), S2_ab = rowsum(E_ab^2), d_ab[p] = f_a[p].f_b[p]:

  D_ab  = S0_aa + S_ab      (S0_aa: diag-zeroed; (a,a): D = 2*S0_aa+e^10)
  half  = 10*d - ln(D) - [N1*R + N2*R^2/2] - ln(1 - exp(10 d)*R)
          where R = 1/D, N_k = Sk0_aa + Sk_ab   (ln(1-x) ~ -(x+x^2/2);
          the x^3 tail is ~1e-6 relative after the alpha/1024 scaling)
  (a,a) pair = 2*(10*d - lnD - [N1*R + N2*R^2/2]), N_k = 2*Sk0_aa
          (the cross-diag term cancels -ln(1-pmt) exactly)

Work sharing: only the 36 unordered blocks are exponentiated. Core c
computes blocks (c, c+j mod 8) for j=0..4 (every unordered pair {a,b}
appears: distance k<=4 on core a, else distance 8-k on core b; the four
distance-4 pairs are computed twice, once per endpoint). Each block
yields BOTH directions' sums:
  rowsum  S_{c,c+j}   : ScalarE Exp accum_out / VectorE stt accum_out
  colsum  S_{c+j,c}   : PE matmul with a ones lhsT, accumulated over the
                        four row-chunks in a [1,512] PSUM bank (only
                        needed for j in {1,2,3}; distance-4 halves use
                        each endpoint's own rowsum)
The host reroutes these O(rows) vectors between cores and applies the
closed-form combine; CE rowsums (exp-accum + iota/is_equal gather) ride
along. All O(N^2) work (matmuls, exp, squares) stays on device.

SPMD: one NEFF for all cores, written for canonical class 0; the host
rotates each core's feature inputs so its own class is block 0 and the
partners are blocks 1..4.
"""

from contextlib import ExitStack

import numpy as np

FLIP = 8
B = 512
D = 128
C = 400
N = 4096
ALPHA = 0.03
E10 = float(np.exp(np.float32(10.0)))
NJ = 5  # partner blocks per core (distances 0..4)

_CACHE = {}


def _build_nc(ebufs=8, sbufs=8, pbufs=4, gbufs=4, cbufs=2):
    import concourse.tile as tile
    from concourse import bacc, mybir

    f32 = mybir.dt.float32
    bf16 = mybir.dt.bfloat16
    AF = mybir.ActivationFunctionType
    OP = mybir.AluOpType
    AX = mybir.AxisListType

    nc = bacc.Bacc("TRN2", target_bir_lowering=False, debug=False)

    ft_d = nc.dram_tensor("ft", [D, NJ * B], bf16, kind="ExternalInput")
    fr_d = nc.dram_tensor("fr", [4, 128, NJ, D], bf16, kind="ExternalInput")
    pred_d = nc.dram_tensor("pred", [B, C], f32, kind="ExternalInput")
    lab_d = nc.dram_tensor("lab", [B, 1], f32, kind="ExternalInput")
    iota_d = nc.dram_tensor("iota", [128, C], f32, kind="ExternalInput")
    eye_d = nc.dram_tensor("eye40", [128, 128], f32, kind="ExternalInput")
    m1_d = nc.dram_tensor("m1", [128, 20], f32, kind="ExternalOutput")
    m2_d = nc.dram_tensor("m2", [128, 20], f32, kind="ExternalOutput")
    dt_d = nc.dram_tensor("dt", [128, 20], f32, kind="ExternalOutput")
    cs1_d = nc.dram_tensor("cs1", [NJ, B], f32, kind="ExternalOutput")
    cs2_d = nc.dram_tensor("cs2", [NJ, B], f32, kind="ExternalOutput")
    ce_d = nc.dram_tensor("ce", [128, 8], f32, kind="ExternalOutput")

    with tile.TileContext(nc) as tc, ExitStack() as ctx:
        const = ctx.enter_context(tc.tile_pool(name="const", bufs=1))
        gpool = ctx.enter_context(tc.tile_pool(name="gp", bufs=gbufs, space="PSUM"))
        cpool = ctx.enter_context(tc.tile_pool(name="cp", bufs=cbufs, space="PSUM"))
        epool = ctx.enter_context(tc.tile_pool(name="ep", bufs=ebufs))
        spool = ctx.enter_context(tc.tile_pool(name="sp", bufs=sbufs))
        fpool = ctx.enter_context(tc.tile_pool(name="fp", bufs=2))
        ppool = ctx.enter_context(tc.tile_pool(name="pp", bufs=pbufs))
        small = ctx.enter_context(tc.tile_pool(name="sm", bufs=1))

        ftt = const.tile([D, NJ * B], bf16)
        iott = const.tile([128, C], f32)
        eyet = const.tile([128, 128], f32)
        ones = const.tile([128, 1], bf16)
        nc.vector.memset(ones[:], 1.0)
        M1 = small.tile([128, 20], f32)
        M2 = small.tile([128, 20], f32)
        dte = small.tile([128, 20], f32)
        cet = small.tile([128, 8], f32)

        # All input DMAs up front, hand-ordered: first ft block 0 (unblocks
        # the Gram pipeline), then the CE inputs, then the rest.
        pred4 = const.tile([128, 4, C], f32)
        lab4 = const.tile([128, 4], f32)
        fr_all = const.tile([128, 4, NJ, D], bf16)
        nc.sync.dma_start(ftt[:, 0:B], ft_d[:, 0:B])
        nc.sync.dma_start(lab4[:], lab_d[:, :].rearrange("(c p) k -> p (c k)", p=128))
        nc.sync.dma_start(iott[:], iota_d[:, :])
        nc.sync.dma_start(pred4[:], pred_d[:, :].rearrange("(c p) k -> p c k", p=128))
        nc.sync.dma_start(eyet[:], eye_d[:, :])
        for j in range(1, NJ):
            nc.sync.dma_start(ftt[:, j * B : (j + 1) * B], ft_d[:, j * B : (j + 1) * B])
        nc.sync.dma_start(fr_all[:], fr_d[:, :, :, :].rearrange("r p j k -> p r j k"))

        # ---- CE loss rowsums (this core's 512 rows of predicts) ----
        for c in range(4):
            mask = ppool.tile([128, C], f32)
            nc.vector.tensor_scalar(
                mask[:], iott[:], lab4[:, c : c + 1], None, OP.is_equal
            )
            scr = ppool.tile([128, C], f32)
            nc.vector.scalar_tensor_tensor(
                scr[:], mask[:], 1.0, pred4[:, c, :], OP.mult, OP.mult,
                accum_out=cet[:, 4 + c : 5 + c],
            )
            scr2 = ppool.tile([128, C], f32)
            nc.scalar.activation(
                scr2[:], pred4[:, c, :], AF.Exp, bias=0.0, scale=1.0,
                accum_out=cet[:, c : c + 1],
            )

        # ---- d_ab[p] = f_a[row] . f_b[row], all partners at once ----
        dvw = dte[:].rearrange("p (j r) -> p j r", r=4)
        for r in range(4):
            frt = fr_all[:, r, :, :]
            prod = fpool.tile([128, NJ, D], f32)
            nc.gpsimd.tensor_tensor(
                prod[:], frt, frt[:, 0:1, :].to_broadcast([128, NJ, D]), OP.mult
            )
            nc.vector.tensor_reduce(
                dvw[:, :, r], prod[:], axis=AX.X, op=OP.add
            )

        # ---- Gram blocks + moments (j outer so colsum PSUM accums are
        #      only live within one j iteration) ----
        for j in range(NJ):
            need_cs = j in (1, 2, 3)
            if need_cs:
                cs1t = cpool.tile([1, B], f32, tag="cs1t")
                cs2t = cpool.tile([1, B], f32, tag="cs2t")
            for r in range(4):
                cidx = j * 4 + r
                gt = gpool.tile([128, B], f32)
                nc.tensor.matmul(
                    gt[:],
                    ftt[:, r * 128 : (r + 1) * 128],
                    ftt[:, j * B : (j + 1) * B],
                    start=True,
                    stop=True,
                )
                if j == 0:
                    # own-block diag: g_pp(=1) -> g_pp-40 so exp(k*10*g)=0
                    nc.vector.tensor_sub(
                        gt[:, r * 128 : (r + 1) * 128],
                        gt[:, r * 128 : (r + 1) * 128],
                        eyet[:],
                    )
                et = epool.tile([128, B], bf16)
                nc.scalar.activation(
                    et[:], gt[:], AF.Exp, bias=0.0, scale=10.0,
                    accum_out=M1[:, cidx : cidx + 1],
                )
                e2 = spool.tile([128, B], bf16)
                nc.vector.scalar_tensor_tensor(
                    e2[:], et[:], 1.0, et[:], OP.mult, OP.mult,
                    accum_out=M2[:, cidx : cidx + 1],
                )
                if need_cs:
                    nc.tensor.matmul(
                        cs1t[:], ones[:], et[:],
                        start=(r == 0), stop=(r == 3),
                    )
                    nc.tensor.matmul(
                        cs2t[:], ones[:], e2[:],
                        start=(r == 0), stop=(r == 3),
                    )
            if need_cs:
                # PSUM can't DMA directly; stage via SBUF (ACT/DVE split)
                cs1s = spool.tile([1, B], f32, tag="cs1s")
                nc.scalar.copy(cs1s[:], cs1t[:])
                nc.sync.dma_start(cs1_d[j : j + 1, :], cs1s[:])
                cs2s = spool.tile([1, B], f32, tag="cs2s")
                nc.vector.tensor_copy(cs2s[:], cs2t[:])
                nc.sync.dma_start(cs2_d[j : j + 1, :], cs2s[:])

        nc.sync.dma_start(m1_d[:, :], M1[:])
        nc.sync.dma_start(m2_d[:, :], M2[:])
        nc.sync.dma_start(dt_d[:, :], dte[:])
        nc.sync.dma_start(ce_d[:, :], cet[:])

    nc.compile()
    return nc


def _get_nc(**kw):
    key = tuple(sorted(kw.items()))
    if key not in _CACHE:
        _CACHE[key] = _build_nc(**kw)
    return _CACHE[key]


def _prep_in_maps(predicts, labels, features):
    import ml_dtypes

    feats = np.ascontiguousarray(features, dtype=np.float32)
    pred = np.ascontiguousarray(predicts, dtype=np.float32)
    lab = np.asarray(labels).astype(np.float32).reshape(N, 1)
    f8 = feats.reshape(B, FLIP, D).transpose(1, 0, 2)  # [8,512,128], f8[c]=feats[c::8]
    iota = np.ascontiguousarray(
        np.broadcast_to(np.arange(C, dtype=np.float32), (128, C))
    )
    eye40 = (40.0 * np.eye(128)).astype(np.float32)
    in_maps = []
    for a in range(FLIP):
        order = [(a + i) % FLIP for i in range(NJ)]
        fo = f8[order]  # [5, 512, 128]: own class then distance 1..4 partners
        ft = np.ascontiguousarray(fo.transpose(2, 0, 1).reshape(D, NJ * B)).astype(
            ml_dtypes.bfloat16
        )
        fr = np.ascontiguousarray(
            fo.transpose(1, 0, 2).reshape(4, 128, NJ, D)
        ).astype(ml_dtypes.bfloat16)
        in_maps.append(
            {
                "ft": ft,
                "fr": fr,
                "pred": np.ascontiguousarray(pred[a * B : (a + 1) * B]),
                "lab": np.ascontiguousarray(lab[a * B : (a + 1) * B]),
                "iota": iota,
                "eye40": eye40,
            }
        )
    return in_maps


def _combine(outs):
    """Host-side O(rows) combine: reroute per-block sums between the
    ordered halves, apply the closed-form series, sum the partials."""
    S1 = {}
    S2 = {}
    dv = {}
    for c in range(FLIP):
        m1 = np.asarray(outs[c]["m1"], np.float64)  # [128, 20] cols j*4+r
        m2 = np.asarray(outs[c]["m2"], np.float64)
        dt = np.asarray(outs[c]["dt"], np.float64)
        cs1 = np.asarray(outs[c]["cs1"], np.float64)  # [5, 512], rows 1..3 used
        cs2 = np.asarray(outs[c]["cs2"], np.float64)
        for j in range(NJ):
            b = (c + j) % FLIP
            # rowsum vectors over rows of f_c: chunk r -> rows 128r..128r+127
            S1[(c, b)] = m1[:, j * 4 : (j + 1) * 4].T.reshape(B)
            S2[(c, b)] = m2[:, j * 4 : (j + 1) * 4].T.reshape(B)
            dv[(c, b)] = dt[:, j * 4 : (j + 1) * 4].T.reshape(B)
            dv[(b, c)] = dv[(c, b)]  # d is batch-indexed, symmetric in (a,b)
        for j in (1, 2, 3):
            # colsum of block (c, c+j) = rowsum of block (c+j, c)
            S1[((c + j) % FLIP, c)] = cs1[j]
            S2[((c + j) % FLIP, c)] = cs2[j]

    nce = 0.0
    for a in range(FLIP):
        S10 = S1[(a, a)]  # diag-zeroed own-block rowsum
        S20 = S2[(a, a)]
        for b in range(FLIP):
            d = dv[(a, b)]
            if a == b:
                N1 = 2.0 * S10
                N2 = 2.0 * S20
                Dv = N1 + E10
                R = 1.0 / Dv
                half = 10.0 * d - np.log(Dv) - (N1 * R + 0.5 * N2 * R * R)
                nce += 2.0 * half.sum()
            else:
                N1 = S10 + S1[(a, b)]
                N2 = S20 + S2[(a, b)]
                Dv = N1
                R = 1.0 / Dv
                half = (
                    10.0 * d
                    - np.log(Dv)
                    - (N1 * R + 0.5 * N2 * R * R)
                    - np.log1p(-np.exp(10.0 * d) * R)
                )
                nce += half.sum()

    ce = 0.0
    for c in range(FLIP):
        se = np.asarray(outs[c]["ce"], np.float64)[:, 0:4]
        xlab = np.asarray(outs[c]["ce"], np.float64)[:, 4:8]
        ce += (np.log(se) - xlab).sum()
    val = ALPHA * (-(nce) / 1024.0) + ce / N
    return np.array(val, dtype=np.float32)


def _run_hw(in_maps, trace=False):
    from concourse.bass_utils import run_bass_kernel_spmd

    nc = _get_nc()
    res = run_bass_kernel_spmd(nc, in_maps, core_ids=list(range(FLIP)), trace=trace)
    return res


def kernel(predicts, labels, features, indexs=None, **_):
    in_maps = _prep_in_maps(predicts, labels, features)
    res = _run_hw(in_maps)
    return _combine(res.results)


def kernel_sim(predicts, labels, features, indexs=None, **_):
    """CoreSim (CPU simulator) path for fast correctness iteration."""
    from concourse.bass_interp import CoreSim

    nc = _get_nc()
    in_maps = _prep_in_maps(predicts, labels, features)
    outs = []
    for a in range(FLIP):
        sim = CoreSim(nc, trace=False)
        for k, v in in_maps[a].items():
            sim.tensor(k)[:] = v
        sim.simulate()
        outs.append(
            {
                k: np.array(sim.tensor(k))
                for k in ("m1", "m2", "dt", "cs1", "cs2", "ce")
            }
        )
    return _combine(outs)



# revision 2
# speedup vs baseline: 1.8713x; 1.8713x over previous
"""Trainium2 Bass kernel for nn_BatchFlipLoss (NCE batch-flip loss + CE loss).

Restructured from the 32.5us baseline around the TRN2 cost model; ~17.4us.

Math (validated to rel-err ~1.9e-5 vs the jax reference; gate is 2e-2):
  The 36-pair NCE sum decomposes per ordered half (a,b) with
  E_ab = exp(10 G_ab), S_ab = rowsum(E_ab), d_ab[p] = f_a[p].f_b[p]:
    cross half = 10 d - ln(N1) - 1 - ln(1 - exp(10 d)/N1),  N1 = S0_aa + S_ab
    self pair  = 2*(10 - ln(D) - N1/D),  N1 = 2 S0_aa, D = N1 + e^10
  The quadratic series term (S2 = rowsum(E^2), ~6e-5 relative) is dropped.

Work split: 36 unordered blocks over 8 cores = 4.5 each. Core c owns
blocks (c, c+j) j=0..3; each distance-4 pair {p, p+4} is split by A-rows
(core p computes E rows 0:256, core p+4 rows 256:512 via host-staged lhsT).
The self block is symmetric: only upper-triangle column slices [128r:512]
are exponentiated; lower-half contributions come from tri colsums.

Device pipeline (one SPMD program, inputs host-rotated per core):
  PE:  bf16 Gram chunk matmuls into 3 cycling 2-bank PSUM groups (p-state
       warmup matmuls run during the input DMAs); -40*I accumulated onto
       self-block diagonals; one-hot-weighted ones-matmuls accumulate all
       column sums + d colsums into one zero-initialized PSUM bank [12,512].
  Act: exp(10g) fused per PSUM group, bf16 out; the only user of ScalarE.
  DVE: per-chunk rowsums via tensor_scalar accum (bf16 4x fast mode); d
       products (own*partner); CE via Schraudolph fast-exp on fp16 logits
       (int32(A*x+B) write, bitcast-f32 rowsum accums).
Host combine: O(rows) rerouting of row/col sums between cores, closed-form
series, CE label-logit gather, final scalar.
"""

from contextlib import ExitStack

import numpy as np

FLIP = 8
B = 512
D = 128
C = 400
N = 4096
ALPHA = 0.03
E10 = float(np.exp(np.float64(10.0)))
NJ = 5

_CACHE = {}

# chunk table: (lhsT kind, lhsT idx, rhs j-slot, m1 col, cs row, rhs off, width)
# lhsT kind "own": ft[:, idx*128:(idx+1)*128]; "j4w": j4w[:, idx*128:...]
# j0 (self block) is symmetric: only the upper-triangle column slice
# [128r:512] is computed per row-chunk r; the lower-half contributions are
# reconstructed on host from the tri colsums (cst rows 8..11).
_CHUNKS = {
    # j0r3's colsum is consumed by no other chunk -> csr None, can be the
    # tail group with only a 128-wide rowsum after the last exp
    "j0": [
        ("own", r, 0, r, (8 + r if r < 3 else None), 128 * r, 512 - 128 * r)
        for r in range(4)
    ],
    "j1": [("own", r, 1, 4 + r, 0, 0, B) for r in range(4)],
    "j2": [("own", r, 2, 8 + r, 1, 0, B) for r in range(4)],
    "j3": [("own", r, 3, 12 + r, 2, 0, B) for r in range(4)],
    "j4": [("j4w", c, 4, 16 + c, 3, 0, B) for c in range(2)],
}
# groups of chunks cycling through 3 two-bank PSUM pools (3-deep PE->Act
# pipeline); final group is a single 128-wide slice so the
# exp->rowsum->DMA tail is minimal.
_GROUPS = [
    _CHUNKS["j1"][0:1],  # single chunks first: earliest possible exp start
    _CHUNKS["j1"][1:2],
    _CHUNKS["j1"][2:4],
    _CHUNKS["j2"][0:2],
    _CHUNKS["j2"][2:4],
    _CHUNKS["j3"][0:2],
    _CHUNKS["j3"][2:4],
    _CHUNKS["j0"][0:2],  # 512 + 384
    _CHUNKS["j4"],
    _CHUNKS["j0"][2:3],  # 256
    _CHUNKS["j0"][3:4],  # 128 (no colsum)
]
_NCS = 14 + 4 + 3  # cross-block + d + j0-tri colsum matmuls

# Schraudolph fast-exp constants for the CE path (exp(x) ~ bitcast_f32
# of int32(A*x + B)); B tuned zero-mean on the CE estimate, robust to
# trunc-vs-round int conversion (validated 6.5e-4 absolute on ce).
SCH_A = float(2**23 / np.log(2))
SCH_B = float(127 * 2**23 - 475000)


def _build_nc():
    import concourse.tile as tile
    from concourse import bacc, mybir

    f32 = mybir.dt.float32
    bf16 = mybir.dt.bfloat16
    f16 = mybir.dt.float16
    AF = mybir.ActivationFunctionType
    OP = mybir.AluOpType

    nc = bacc.Bacc("TRN2", target_bir_lowering=False, debug=False)

    ft_d = nc.dram_tensor("ft", [D, NJ * B], bf16, kind="ExternalInput")
    j4w_d = nc.dram_tensor("j4w", [D, 256], bf16, kind="ExternalInput")
    pred_d = nc.dram_tensor("pred", [128, 4 * C], f16, kind="ExternalInput")
    eye_d = nc.dram_tensor("eye2", [128, 2, 128], bf16, kind="ExternalInput")
    oh_d = nc.dram_tensor("oh", [128, 144], bf16, kind="ExternalInput")
    m1_d = nc.dram_tensor("m1", [128, 22], f32, kind="ExternalOutput")
    cs_d = nc.dram_tensor("cs", [12, B], f32, kind="ExternalOutput")

    with tile.TileContext(nc) as tc, ExitStack() as ctx:
        const = ctx.enter_context(tc.tile_pool(name="const", bufs=1))
        pg = [
            ctx.enter_context(tc.tile_pool(name=f"pg{i}", bufs=1, space="PSUM"))
            for i in range(3)
        ]
        pwu = ctx.enter_context(tc.tile_pool(name="pwu", bufs=1, space="PSUM"))
        pcs = ctx.enter_context(tc.tile_pool(name="pcs", bufs=1, space="PSUM"))
        pet = ctx.enter_context(tc.tile_pool(name="pet", bufs=4))
        pscr = ctx.enter_context(tc.tile_pool(name="pscr", bufs=2))
        small = ctx.enter_context(tc.tile_pool(name="small", bufs=1))

        ftt = const.tile([D, NJ * B], bf16)
        j4wt = const.tile([D, 256], bf16)
        predt = const.tile([128, 4 * C], f16)
        eyet = const.tile([128, 2, 128], bf16)
        oht = const.tile([128, 144], bf16)
        M1 = small.tile([128, 22], f32)
        cs_s = small.tile([12, B], f32)
        ce_i32 = small.tile([128, 4 * C], mybir.dt.int32)
        db = small.tile([128, 4, B], bf16)

        # input DMAs in Gram-pipeline priority order: own+j1 block first
        # (unblocks fills 0-2), then j2, then j3+j4rhs, then the rest —
        # large DMAs occupy all engines sequentially, so order is latency.
        nc.sync.dma_start(ftt[:, 0 : 2 * B], ft_d[:, 0 : 2 * B])
        nc.sync.dma_start(oht[:], oh_d[:, :])  # tiny; colsums need it early
        nc.sync.dma_start(ftt[:, 2 * B : 3 * B], ft_d[:, 2 * B : 3 * B])
        nc.sync.dma_start(ftt[:, 3 * B :], ft_d[:, 3 * B :])
        nc.sync.dma_start(eyet[:], eye_d[:, :])
        nc.sync.dma_start(j4wt[:], j4w_d[:, :])
        nc.sync.dma_start(predt[:], pred_d[:, :])

        # colsum accumulator bank: rows 0-2 cs j1-3, 3 cs j4, 4-7 d j1-4,
        # 8-10 j0 tri colsums (row 8+r holds block-cols 128r.. at offset 0).
        # Zero-initialized so every colsum matmul can accumulate with
        # start=False — the scheduler may reorder accumulating matmuls, so
        # no single one can safely carry the start flag.
        cst = pcs.tile([12, B], f32)
        nc.vector.memset(cst[:], 0.0)

        # ---- PE p-state warmup: dummy matmuls on a memset tile while the
        # input DMAs land, so real matmuls start at full clock (the Tensor
        # engine needs ~3us of continuous execution to leave mid p-state).
        # Dedicated PSUM bank so no WAW dependency delays the real fills.
        wu = const.tile([128, B], bf16)
        nc.gpsimd.memset(wu[:], 0.0625)
        warm = pwu.tile([128, B], f32)
        for i in range(5):
            nc.tensor.matmul(
                warm[:, :],
                wu[:, 0:128],
                wu[:],
                start=True,
                stop=True,
                skip_group_check=True,
            )



        # ---- Gram pipeline ----
        ngroups = len(_GROUPS)
        ets = [None] * ngroups
        gts = [None] * ngroups
        spans = [None] * ngroups

        def _offsets(chunks):
            offs, o = [], 0
            for ch in chunks:
                offs.append(o)
                o += ch[6]
            return offs, o

        def fill_group(gi):
            chunks = _GROUPS[gi]
            offs, w = _offsets(chunks)
            pool = pg[gi % 3]
            gt = pool.tile([128, 1024], f32, tag=f"g{gi % 3}")
            for (kind, idx, j, m1c, csr, roff, width), o in zip(chunks, offs):
                lhsT = (
                    ftt[:, idx * 128 : (idx + 1) * 128]
                    if kind == "own"
                    else j4wt[:, idx * 128 : (idx + 1) * 128]
                )
                nc.tensor.matmul(
                    gt[:, o : o + width],
                    lhsT,
                    ftt[:, j * B + roff : j * B + roff + width],
                    start=True,
                    stop=(j != 0),
                    skip_group_check=(j == 0),
                )
                if j == 0:
                    # own-block diag: accumulate -40*I so exp(10(g-40)) -> 0
                    # (diag sub-block is the first 128 cols of the tri slice)
                    nc.tensor.matmul(
                        gt[:, o : o + 128],
                        eyet[:, 0, :],
                        eyet[:, 1, :],
                        start=False,
                        stop=True,
                        skip_group_check=True,
                    )
            gts[gi] = gt
            spans[gi] = w

        def exp_group(gi):
            w = spans[gi]
            et = pet.tile([128, 1024], bf16, tag="et")
            nc.scalar.activation(
                et[:, 0:w], gts[gi][:, 0:w], AF.Exp, bias=0.0, scale=10.0
            )
            ets[gi] = et

        def sums_group(gi):
            chunks = _GROUPS[gi]
            offs, _ = _offsets(chunks)
            et = ets[gi]
            for (kind, idx, j, m1c, csr, roff, width), o in zip(chunks, offs):
                scr = pscr.tile([128, B], bf16, tag="scr")
                nc.vector.tensor_scalar(
                    scr[:, 0:width],
                    et[:, o : o + width],
                    1.0,
                    None,
                    OP.mult,
                    OP.add,
                    accum_out=M1[:, m1c : m1c + 1],
                )

        # all colsum matmuls form ONE accumulation group into cst [8,512]:
        # lhsT = one-hot column csr of ones -> adds rowsum into row csr
        NCS = _NCS
        cs_count = [0]

        def cs_matmul(csr, rhs, width=B):
            i = cs_count[0]
            cs_count[0] += 1
            nc.tensor.matmul(
                cst[:, 0:width],
                oht[:, csr * 12 : (csr + 1) * 12],
                rhs,
                start=False,
                stop=(i == NCS - 1),
                skip_group_check=True,
            )

        def cs_group(gi):
            chunks = _GROUPS[gi]
            offs, _ = _offsets(chunks)
            et = ets[gi]
            for (kind, idx, j, m1c, csr, roff, width), o in zip(chunks, offs):
                if csr is None:
                    continue
                cs_matmul(csr, et[:, o : o + width], width)

        # PE order: g0, g1, d-colsums, then fill g(i+1) before cs(g i-1)
        fill_group(0)
        exp_group(0)
        fill_group(1)
        exp_group(1)
        fill_group(2)
        exp_group(2)
        sums_group(0)
        sums_group(1)
        for gi in range(3, ngroups):
            fill_group(gi)
            exp_group(gi)
            cs_group(gi - 3)
            sums_group(gi - 1)  # (sums 0,1 issued above)
            # DVE filler work goes after the pipeline-critical sums so the
            # scheduler always prefers sums (they gate et-slot recycling)
            if gi == 3:
                # d products (elementwise own*partner, then PE colsums)
                ftv = ftt[:].rearrange("p (j b) -> p j b", j=NJ)
                nc.vector.tensor_tensor(
                    db[:],
                    ftv[:, 1:NJ, :],
                    ftv[:, 0:1, :].to_broadcast([128, 4, B]),
                    OP.mult,
                )
            elif gi == 4:
                # CE on DVE via Schraudolph fast-exp: int32(A*x+B) then
                # bitcast-f32 rowsums; frees ScalarE for the Gram exps.
                nc.vector.tensor_scalar(
                    ce_i32[:], predt[:], SCH_A, SCH_B, OP.mult, OP.add
                )
            elif gi == 5:
                ce_f32 = ce_i32[:].bitcast(f32)
                for c in range(4):
                    scr2 = pscr.tile([128, B], f32, tag="scr2")
                    nc.vector.tensor_scalar(
                        scr2[:, 0:C],
                        ce_f32[:, c * C : (c + 1) * C],
                        1.0,
                        None,
                        OP.mult,
                        OP.add,
                        accum_out=M1[:, 18 + c : 19 + c],
                    )
        for jj in range(4):
            cs_matmul(4 + jj, db[:, jj, :])
        cs_group(ngroups - 3)  # j4
        cs_group(ngroups - 2)  # j0r2 tri
        sums_group(ngroups - 1)
        nc.sync.dma_start(m1_d[:, :], M1[:])
        # stage colsum bank to SBUF, then DMA out
        nc.vector.tensor_copy(cs_s[:], cst[:])
        nc.sync.dma_start(cs_d[:, :], cs_s[:])

    nc.compile()
    return nc


def _get_nc():
    if "nc" not in _CACHE:
        _CACHE["nc"] = _build_nc()
    return _CACHE["nc"]


def _prep_in_maps(predicts, labels, features):
    import ml_dtypes

    feats = np.ascontiguousarray(features, dtype=np.float32)
    pred = np.ascontiguousarray(predicts, dtype=np.float32)
    f8 = feats.reshape(B, FLIP, D).transpose(1, 0, 2)  # [8,512,128]
    eye2 = np.stack(
        [-40.0 * np.eye(128, dtype=np.float32), np.eye(128, dtype=np.float32)], axis=1
    ).astype(ml_dtypes.bfloat16)  # [128, 2, 128]: lhsT=-40I, rhs=I
    oh = np.zeros((128, 12, 12), dtype=np.float32)
    for r in range(12):
        oh[:, r, r] = 1.0
    oh = oh.reshape(128, 144).astype(ml_dtypes.bfloat16)
    in_maps = []
    for a in range(FLIP):
        order = [(a + i) % FLIP for i in range(NJ)]
        fo = f8[order].copy()  # [5, 512, 128]
        if a >= 4:
            fo[4] = f8[a]  # j4 Gram rhs = own (pair-B side)
        ft = np.ascontiguousarray(fo.transpose(2, 0, 1).reshape(D, NJ * B)).astype(
            ml_dtypes.bfloat16
        )
        pa = a if a < 4 else a - 4
        rows = slice(0, 256) if a < 4 else slice(256, 512)
        j4w = np.ascontiguousarray(f8[pa][rows].T).astype(ml_dtypes.bfloat16)
        pr = (
            pred[a * B : (a + 1) * B]
            .reshape(4, 128, C)
            .transpose(1, 0, 2)
            .reshape(128, 4 * C)
        )
        in_maps.append(
            {
                "ft": ft,
                "j4w": np.ascontiguousarray(j4w),
                "pred": np.ascontiguousarray(pr).astype(np.float16),
                "eye2": np.ascontiguousarray(eye2),
                "oh": oh,
            }
        )
    return in_maps


def _combine(outs, predicts, labels):
    """Host O(rows) combine: reroute per-block sums, closed-form series."""
    S1 = {}
    S10 = {}
    dv = {}
    for c in range(FLIP):
        m1 = np.asarray(outs[c]["m1"], np.float64)  # [128, 22]
        cs = np.asarray(outs[c]["cs"], np.float64)  # [12, 512]
        # self-block from upper-tri slices: rowsum over cols 128r:512 (m1)
        # plus transposed contributions from the tri colsums of chunks r'<r
        s10 = np.empty(B)
        for r in range(4):
            v = m1[:, r].copy()
            for rp in range(r):
                v += cs[8 + rp][128 * (r - rp) : 128 * (r - rp) + 128]
            s10[128 * r : 128 * (r + 1)] = v
        S10[c] = s10
        for j in (1, 2, 3):
            b = (c + j) % FLIP
            S1[(c, b)] = m1[:, j * 4 : (j + 1) * 4].T.reshape(B)
            S1[(b, c)] = cs[j - 1]
            dv[(c, b)] = cs[3 + j]
            dv[(b, c)] = cs[3 + j]
    for p in range(4):
        b = p + 4
        S1[(p, b)] = np.concatenate(
            [
                np.asarray(outs[p]["m1"], np.float64)[:, 16:18].T.reshape(256),
                np.asarray(outs[b]["m1"], np.float64)[:, 16:18].T.reshape(256),
            ]
        )
        S1[(b, p)] = (
            np.asarray(outs[p]["cs"], np.float64)[3]
            + np.asarray(outs[b]["cs"], np.float64)[3]
        )
        d = np.asarray(outs[p]["cs"], np.float64)[7]
        dv[(p, b)] = d
        dv[(b, p)] = d

    nce = 0.0
    for a in range(FLIP):
        for b in range(FLIP):
            if a == b:
                N1 = 2.0 * S10[a]
                Dv = N1 + E10
                half = 10.0 - np.log(Dv) - N1 / Dv
                nce += 2.0 * half.sum()
            else:
                d = dv[(a, b)]
                N1 = S10[a] + S1[(a, b)]
                half = (
                    10.0 * d
                    - np.log(N1)
                    - 1.0
                    - np.log1p(-np.exp(10.0 * d) / N1)
                )
                nce += half.sum()

    # CE: device gives per-row sum(exp(pred)); label logit gathered on host
    se = np.concatenate(
        [np.asarray(outs[c]["m1"], np.float64)[:, 18:22].T.reshape(B) for c in range(FLIP)]
    )
    lab = np.asarray(labels).astype(np.int64)
    pred16 = np.asarray(predicts, np.float32).astype(np.float16)
    xlab = pred16[np.arange(N), lab].astype(np.float64)
    ce = (np.log(se) - xlab).mean()

    val = ALPHA * (-(nce) / 1024.0) + ce
    return np.array(val, dtype=np.float32)


def _run_hw(in_maps, trace=False):
    from concourse.bass_utils import run_bass_kernel_spmd

    nc = _get_nc()
    return run_bass_kernel_spmd(nc, in_maps, core_ids=list(range(FLIP)), trace=trace)


def kernel(predicts, labels, features, indexs=None, **_):
    in_maps = _prep_in_maps(predicts, labels, features)
    res = _run_hw(in_maps)
    return _combine(res.results, predicts, labels)


def kernel_sim(predicts, labels, features, indexs=None, **_):
    """CoreSim (CPU simulator) path for fast correctness iteration."""
    from concourse.bass_interp import CoreSim

    nc = _get_nc()
    in_maps = _prep_in_maps(predicts, labels, features)
    outs = []
    for a in range(FLIP):
        sim = CoreSim(nc, trace=False)
        for k, v in in_maps[a].items():
            sim.tensor(k)[:] = v
        sim.simulate()
        outs.append({k: np.array(sim.tensor(k)) for k in ("m1", "cs")})
    return _combine(outs, predicts, labels)


# revision 4
# speedup vs baseline: 1.9171x; 1.0245x over previous
"""Trainium2 Bass kernel for nn_BatchFlipLoss (NCE batch-flip loss + CE loss).

Restructured from the 32.5us baseline around the TRN2 cost model; ~17.0us.

Math (validated to rel-err ~1.5e-5 vs the jax reference; gate is 2e-2):
  The 36-pair NCE sum decomposes per ordered half (a,b) with
  E_ab = exp(10 G_ab), S_ab = rowsum(E_ab), d_ab[p] = f_a[p].f_b[p]:
    cross half = 10 d - ln(N1) - 1 - ln(1 - exp(10 d)/N1),  N1 = S0_aa + S_ab
    self pair  = 2*(10 - ln(D) - N1/D),  N1 = 2 S0_aa, D = N1 + e^10
  The quadratic series term (S2 = rowsum(E^2), ~6e-5 relative) is dropped.

Work split: 36 unordered blocks over 8 cores = 4.5 each. Core c owns
blocks (c, c+j) j=0..3; each distance-4 pair {p, p+4} is split by A-rows
(core p computes E rows 0:256, core p+4 rows 256:512 via host-staged lhsT).
The self block is symmetric: upper-triangle column slices only; lower-half
contributions are reconstructed on host from tri colsums.

Device pipeline (one SPMD program, inputs host-rotated per core):
  PE:  bf16 Gram chunk matmuls into 3 cycling 2-bank PSUM groups (p-state
       warmup matmuls run during the input DMAs); -8*I accumulated onto
       self-block diagonals; one-hot-weighted ones-matmuls accumulate all
       column sums into one zero-initialized PSUM bank [12,512].
  Act: exp(10g) fused per PSUM group, bf16 out — the only user of ScalarE;
       the last (128-wide) group's rowsum rides the exp accumulator.
  DVE: per-chunk rowsums via tensor_scalar accum (bf16 4x fast mode); d
       products (own*partner); CE and the j0r2 tri slice via Schraudolph
       fast-exp (int32(A*x+B) write, bitcast-f32 read) so both stay off
       the ScalarE critical path.
  Pool: d colsums via partition_all_reduce (partition 0 DMAd mid-stream).
Host combine: O(rows) rerouting of row/col sums between cores, closed-form
series, CE label-logit gather, final scalar.
"""

from contextlib import ExitStack

import numpy as np

FLIP = 8
B = 512
D = 128
C = 400
N = 4096
ALPHA = 0.03
E10 = float(np.exp(np.float64(10.0)))
NJ = 5

_CACHE = {}

# chunk table: (lhsT kind, lhsT idx, rhs j-slot, m1 col, cs row, rhs off, width)
# lhsT kind "own": ft[:, idx*128:(idx+1)*128]; "j4w": j4w[:, idx*128:...]
# j0 (self block) is symmetric: only the upper-triangle column slice
# [128r:512] is computed per row-chunk r; the lower-half contributions are
# reconstructed on host from the tri colsums (cst rows 8..11).
_CHUNKS = {
    # j0r0/r1 are upper-tri slices (their colsums feed r1/r2's host
    # reconstruction); r2 is a tri slice whose colsum nobody consumes
    # (csr None); r3 is computed FULL-width so it needs no reconstruction
    # at all -- r2+r3 form a colsum-free tail, letting the cst bank (and
    # its staging copy + DMA) retire ~1us before the exp stream ends.
    "j0": [
        ("own", 0, 0, 0, 8, 0, B),
        ("own", 1, 0, 1, 9, 128, 384),
        ("own", 2, 0, 2, 10, 256, 256),
        ("own", 3, 0, 3, None, 384, 128),
    ],
    "j1": [("own", r, 1, 4 + r, 0, 0, B) for r in range(4)],
    "j2": [("own", r, 2, 8 + r, 1, 0, B) for r in range(4)],
    "j3": [("own", r, 3, 12 + r, 2, 0, B) for r in range(4)],
    "j4": [("j4w", c, 4, 16 + c, 3, 0, B) for c in range(2)],
}
# 11 groups of <=1024 cycling three 2-bank PSUM pools (3-deep PE->Act
# pipeline): two single-chunk groups first for the earliest exp start,
# the two colsum-free j0 slices last (the cst bank + staging copy retire
# one full group before the exp stream ends).
_GROUPS = [
    _CHUNKS["j1"][0:1],   # 512
    _CHUNKS["j1"][1:2],   # 512
    _CHUNKS["j1"][2:4],   # 1024
    _CHUNKS["j2"][0:2],   # 1024
    _CHUNKS["j2"][2:4],   # 1024
    _CHUNKS["j3"][0:2],   # 1024
    _CHUNKS["j3"][2:4],   # 1024
    _CHUNKS["j0"][0:2],   # 896 (tri r0+r1)
    _CHUNKS["j4"],        # 1024 (last with colsums)
    _CHUNKS["j0"][3:4],   # 128 (tri, no colsum; rowsum via exp accum_out)
]
# j0r2 (tri, 256) is computed OFF the ScalarE stream via a DVE
# Schraudolph fast-exp so its colsum (the cst staging-copy gate) retires
# ~1.5us before the exp stream ends.
_NCS = 14 + 3  # cross-block + j0-tri (r0, r1, r2) colsum matmuls

# Schraudolph fast-exp constants for the CE path (exp(x) ~ bitcast_f32
# of int32(A*x + B)); B tuned zero-mean on the CE estimate, robust to
# trunc-vs-round int conversion (validated 6.5e-4 absolute on ce).
SCH_A = float(2**23 / np.log(2))
SCH_B = float(127 * 2**23 - 475000)


def _build_nc():
    import concourse.tile as tile
    from concourse import bacc, mybir

    f32 = mybir.dt.float32
    bf16 = mybir.dt.bfloat16
    f16 = mybir.dt.float16
    AF = mybir.ActivationFunctionType
    OP = mybir.AluOpType

    nc = bacc.Bacc("TRN2", target_bir_lowering=False, debug=False)

    ft_d = nc.dram_tensor("ft", [D, NJ * B], bf16, kind="ExternalInput")
    j4w_d = nc.dram_tensor("j4w", [D, 256], bf16, kind="ExternalInput")
    pred_d = nc.dram_tensor("pred", [128, 4 * C], f16, kind="ExternalInput")
    eye_d = nc.dram_tensor("eye2", [128, 2, 128], bf16, kind="ExternalInput")
    oh_d = nc.dram_tensor("oh", [128, 144], bf16, kind="ExternalInput")
    m1_d = nc.dram_tensor("m1", [128, 22], f32, kind="ExternalOutput")
    cs_d = nc.dram_tensor("cs", [12, B], f32, kind="ExternalOutput")
    d_d = nc.dram_tensor("dv", [4, B], f32, kind="ExternalOutput")

    with tile.TileContext(nc) as tc, ExitStack() as ctx:
        const = ctx.enter_context(tc.tile_pool(name="const", bufs=1))
        pg = [
            ctx.enter_context(tc.tile_pool(name=f"pg{i}", bufs=1, space="PSUM"))
            for i in range(3)
        ]
        pwu = ctx.enter_context(tc.tile_pool(name="pwu", bufs=1, space="PSUM"))
        pcs = ctx.enter_context(tc.tile_pool(name="pcs", bufs=1, space="PSUM"))
        pet = ctx.enter_context(tc.tile_pool(name="pet", bufs=4))
        pscr = ctx.enter_context(tc.tile_pool(name="pscr", bufs=2))
        small = ctx.enter_context(tc.tile_pool(name="small", bufs=1))

        ftt = const.tile([D, NJ * B], bf16)
        j4wt = const.tile([D, 256], bf16)
        predt = const.tile([128, 4 * C], f16)
        eyet = const.tile([128, 2, 128], bf16)
        oht = const.tile([128, 144], bf16)
        M1 = small.tile([128, 22], f32)
        cs_s = small.tile([12, B], f32)
        ce_i32 = small.tile([128, 4 * C], mybir.dt.int32)
        db = small.tile([128, 4, B], bf16)
        dred = small.tile([128, 4, B], f32)
        sj32 = small.tile([128, 256], mybir.dt.int32)
        sjbf = small.tile([128, 256], bf16)
        wt2h = [None]

        # input DMAs in Gram-pipeline priority order: own+j1 block first
        # (unblocks fills 0-2), then j2, then j3+j4rhs, then the rest —
        # large DMAs occupy all engines sequentially, so order is latency.
        nc.sync.dma_start(ftt[:, 0 : 2 * B], ft_d[:, 0 : 2 * B])
        nc.sync.dma_start(oht[:], oh_d[:, :])  # tiny; colsums need it early
        nc.sync.dma_start(ftt[:, 2 * B : 3 * B], ft_d[:, 2 * B : 3 * B])
        nc.sync.dma_start(ftt[:, 3 * B :], ft_d[:, 3 * B :])
        nc.sync.dma_start(eyet[:], eye_d[:, :])
        nc.sync.dma_start(j4wt[:], j4w_d[:, :])
        nc.sync.dma_start(predt[:], pred_d[:, :])

        # colsum accumulator bank: rows 0-2 cs j1-3, 3 cs j4, 4-7 d j1-4,
        # 8-10 j0 tri colsums (row 8+r holds block-cols 128r.. at offset 0).
        # Zero-initialized so every colsum matmul can accumulate with
        # start=False — the scheduler may reorder accumulating matmuls, so
        # no single one can safely carry the start flag.
        cst = pcs.tile([12, B], f32)
        nc.vector.memset(cst[:], 0.0)

        # ---- PE p-state warmup: dummy matmuls on a memset tile while the
        # input DMAs land, so real matmuls start at full clock (the Tensor
        # engine needs ~3us of continuous execution to leave mid p-state).
        # Dedicated PSUM bank so no WAW dependency delays the real fills.
        wu = const.tile([128, B], bf16)
        nc.gpsimd.memset(wu[:], 0.0625)
        warm = pwu.tile([128, B], f32, tag="wu")
        for i in range(5):
            nc.tensor.matmul(
                warm[:, :],
                wu[:, 0:128],
                wu[:],
                start=True,
                stop=True,
                skip_group_check=True,
            )



        # ---- Gram pipeline ----
        ngroups = len(_GROUPS)
        ets = [None] * ngroups
        gts = [None] * ngroups
        spans = [None] * ngroups

        def _offsets(chunks):
            offs, o = [], 0
            for ch in chunks:
                offs.append(o)
                o += ch[6]
            return offs, o

        def fill_group(gi):
            chunks = _GROUPS[gi]
            offs, w = _offsets(chunks)
            pool = pg[gi % 3]
            gt = pool.tile([128, 1024], f32, tag=f"g{gi % 3}")
            for (kind, idx, j, m1c, csr, roff, width), o in zip(chunks, offs):
                lhsT = (
                    ftt[:, idx * 128 : (idx + 1) * 128]
                    if kind == "own"
                    else j4wt[:, idx * 128 : (idx + 1) * 128]
                )
                nc.tensor.matmul(
                    gt[:, o : o + width],
                    lhsT,
                    ftt[:, j * B + roff : j * B + roff + width],
                    start=True,
                    stop=(j != 0),
                    skip_group_check=(j == 0),
                )
                if j == 0:
                    # own-block diag: accumulate -8*I; exp(10(g-8)) ~ 4e-31
                    # (negligible in the sums; -8 keeps the Schraudolph
                    # affine for the DVE j0r2 path positive in int32)
                    dg = o + idx * 128 - roff
                    nc.tensor.matmul(
                        gt[:, dg : dg + 128],
                        eyet[:, 0, :],
                        eyet[:, 1, :],
                        start=False,
                        stop=True,
                        skip_group_check=True,
                    )
            gts[gi] = gt
            spans[gi] = w

        def exp_group(gi, accum_m1c=None):
            w = spans[gi]
            et = pet.tile([128, 1024], bf16, tag="et")
            kw = {}
            if accum_m1c is not None:
                # last group: the rowsum rides the exp's own accumulator
                # (+187ns on ScalarE) instead of a DVE pass that would race
                # the cst staging copy at the tail
                kw["accum_out"] = M1[:, accum_m1c : accum_m1c + 1]
            nc.scalar.activation(
                et[:, 0:w], gts[gi][:, 0:w], AF.Exp, bias=0.0, scale=10.0, **kw
            )
            ets[gi] = et

        def sums_group(gi):
            chunks = _GROUPS[gi]
            offs, _ = _offsets(chunks)
            et = ets[gi]
            for (kind, idx, j, m1c, csr, roff, width), o in zip(chunks, offs):
                scr = pscr.tile([128, B], bf16, tag="scr")
                nc.vector.tensor_scalar(
                    scr[:, 0:width],
                    et[:, o : o + width],
                    1.0,
                    None,
                    OP.mult,
                    OP.add,
                    accum_out=M1[:, m1c : m1c + 1],
                )

        # all colsum matmuls form ONE accumulation group into cst [8,512]:
        # lhsT = one-hot column csr of ones -> adds rowsum into row csr
        NCS = _NCS
        cs_count = [0]

        def cs_matmul(csr, rhs, width=B):
            i = cs_count[0]
            cs_count[0] += 1
            nc.tensor.matmul(
                cst[:, 0:width],
                oht[:, csr * 12 : (csr + 1) * 12],
                rhs,
                start=False,
                stop=(i == NCS - 1),
                skip_group_check=True,
            )

        def cs_group(gi):
            chunks = _GROUPS[gi]
            offs, _ = _offsets(chunks)
            et = ets[gi]
            for (kind, idx, j, m1c, csr, roff, width), o in zip(chunks, offs):
                if csr is None:
                    continue
                cs_matmul(csr, et[:, o : o + width], width)

        # PE order: g0, g1, d-colsums, then fill g(i+1) before cs(g i-1)
        fill_group(0)
        exp_group(0)
        fill_group(1)
        exp_group(1)
        fill_group(2)
        exp_group(2)
        sums_group(0)
        sums_group(1)
        for gi in range(3, ngroups):
            fill_group(gi)
            if gi == ngroups - 1:
                exp_group(gi, accum_m1c=_GROUPS[gi][0][3])
            else:
                exp_group(gi)
            cs_group(gi - 3)
            sums_group(gi - 1)  # (sums 0,1 issued above)
            # DVE filler work goes after the pipeline-critical sums so the
            # scheduler always prefers sums (they gate et-slot recycling)
            if gi == 3:
                # d products (elementwise own*partner on DVE); the column
                # sums run on the otherwise-idle GPSIMD engine (result is
                # partition-broadcast, partition 0 is DMAd out mid-stream)
                # so neither PE nor the cst staging copy is involved.
                ftv = ftt[:].rearrange("p (j b) -> p j b", j=NJ)
                nc.vector.tensor_tensor(
                    db[:],
                    ftv[:, 1:NJ, :],
                    ftv[:, 0:1, :].to_broadcast([128, 4, B]),
                    OP.mult,
                )
                from concourse import bass_isa

                nc.gpsimd.partition_all_reduce(
                    dred[:], db[:], 128, bass_isa.ReduceOp.add
                )
                nc.sync.dma_start(d_d[:, :], dred[0:1, :, :])
            elif gi == 4:
                # CE on DVE via Schraudolph fast-exp: int32(A*x+B) then
                # bitcast-f32 rowsums; frees ScalarE for the Gram exps.
                nc.vector.tensor_scalar(
                    ce_i32[:], predt[:], SCH_A, SCH_B, OP.mult, OP.add
                )
            elif gi == 5:
                ce_f32 = ce_i32[:].bitcast(f32)
                for c in range(4):
                    scr2 = pscr.tile([128, B], f32, tag="scr2")
                    nc.vector.tensor_scalar(
                        scr2[:, 0:C],
                        ce_f32[:, c * C : (c + 1) * C],
                        1.0,
                        None,
                        OP.mult,
                        OP.add,
                        accum_out=M1[:, 18 + c : 19 + c],
                    )
            elif gi == 6:
                # j0r2 Gram into the spare warmup bank (PE slack window)
                wt2 = pwu.tile([128, B], f32, tag="wu")
                wt2h[0] = wt2
                nc.tensor.matmul(
                    wt2[:, 0:256],
                    ftt[:, 256:384],
                    ftt[:, 256:512],
                    start=True,
                    stop=False,
                    skip_group_check=True,
                )
                nc.tensor.matmul(
                    wt2[:, 0:128],
                    eyet[:, 0, :],
                    eyet[:, 1, :],
                    start=False,
                    stop=True,
                    skip_group_check=True,
                )
            elif gi == 7:
                # j0r2 exp via DVE Schraudolph: int32(10A*g + B), bitcast
                # f32 -> bf16 with the rowsum riding the accum (-8-shifted
                # diag keeps the affine positive; residual ~2^-101)
                nc.vector.tensor_scalar(
                    sj32[:],
                    wt2h[0][:, 0:256],
                    10.0 * SCH_A,
                    SCH_B,
                    OP.mult,
                    OP.add,
                )
                nc.vector.tensor_scalar(
                    sjbf[:],
                    sj32[:].bitcast(f32),
                    1.0,
                    None,
                    OP.mult,
                    OP.add,
                    accum_out=M1[:, 2:3],
                )
        cs_matmul(10, sjbf[:, 0:256], 256)  # j0r2 tri colsum (ready early)
        cs_group(ngroups - 3)  # j0 tri r0+r1 colsums
        cs_group(ngroups - 2)  # j4 colsums
        nc.sync.dma_start(m1_d[:, :], M1[:])
        # stage colsum bank to SBUF, then DMA out
        nc.vector.tensor_copy(cs_s[:], cst[:])
        nc.sync.dma_start(cs_d[:, :], cs_s[:])

    nc.compile()
    return nc


def _get_nc():
    if "nc" not in _CACHE:
        _CACHE["nc"] = _build_nc()
    return _CACHE["nc"]


def _prep_in_maps(predicts, labels, features):
    import ml_dtypes

    feats = np.ascontiguousarray(features, dtype=np.float32)
    pred = np.ascontiguousarray(predicts, dtype=np.float32)
    f8 = feats.reshape(B, FLIP, D).transpose(1, 0, 2)  # [8,512,128]
    eye2 = np.stack(
        [-8.0 * np.eye(128, dtype=np.float32), np.eye(128, dtype=np.float32)], axis=1
    ).astype(ml_dtypes.bfloat16)  # [128, 2, 128]: lhsT=-8I, rhs=I
    oh = np.zeros((128, 12, 12), dtype=np.float32)
    for r in range(12):
        oh[:, r, r] = 1.0
    oh = oh.reshape(128, 144).astype(ml_dtypes.bfloat16)
    in_maps = []
    for a in range(FLIP):
        order = [(a + i) % FLIP for i in range(NJ)]
        fo = f8[order].copy()  # [5, 512, 128]
        if a >= 4:
            fo[4] = f8[a]  # j4 Gram rhs = own (pair-B side)
        ft = np.ascontiguousarray(fo.transpose(2, 0, 1).reshape(D, NJ * B)).astype(
            ml_dtypes.bfloat16
        )
        pa = a if a < 4 else a - 4
        rows = slice(0, 256) if a < 4 else slice(256, 512)
        j4w = np.ascontiguousarray(f8[pa][rows].T).astype(ml_dtypes.bfloat16)
        pr = (
            pred[a * B : (a + 1) * B]
            .reshape(4, 128, C)
            .transpose(1, 0, 2)
            .reshape(128, 4 * C)
        )
        in_maps.append(
            {
                "ft": ft,
                "j4w": np.ascontiguousarray(j4w),
                "pred": np.ascontiguousarray(pr).astype(np.float16),
                "eye2": np.ascontiguousarray(eye2),
                "oh": oh,
            }
        )
    return in_maps


def _combine(outs, predicts, labels):
    """Host O(rows) combine: reroute per-block sums, closed-form series."""
    S1 = {}
    S10 = {}
    dv = {}
    for c in range(FLIP):
        m1 = np.asarray(outs[c]["m1"], np.float64)  # [128, 22]
        cs = np.asarray(outs[c]["cs"], np.float64)  # [12, 512]
        dvr = np.asarray(outs[c]["dv"], np.float64)  # [4, 512]
        # self-block from upper-tri slices: rowsum over cols 128r:512 (m1)
        # plus transposed contributions from the tri colsums of chunks r'<r
        # (cst row 8+r' holds slice-r' colsums at offset -128r')
        s10 = np.empty(B)
        for r in range(4):
            v = m1[:, r].copy()
            for rp in range(r):
                v += cs[8 + rp][128 * (r - rp) : 128 * (r - rp) + 128]
            s10[128 * r : 128 * (r + 1)] = v
        S10[c] = s10
        for j in (1, 2, 3):
            b = (c + j) % FLIP
            S1[(c, b)] = m1[:, j * 4 : (j + 1) * 4].T.reshape(B)
            S1[(b, c)] = cs[j - 1]
            dv[(c, b)] = dvr[j - 1]
            dv[(b, c)] = dvr[j - 1]
    for p in range(4):
        b = p + 4
        S1[(p, b)] = np.concatenate(
            [
                np.asarray(outs[p]["m1"], np.float64)[:, 16:18].T.reshape(256),
                np.asarray(outs[b]["m1"], np.float64)[:, 16:18].T.reshape(256),
            ]
        )
        S1[(b, p)] = (
            np.asarray(outs[p]["cs"], np.float64)[3]
            + np.asarray(outs[b]["cs"], np.float64)[3]
        )
        d = np.asarray(outs[p]["dv"], np.float64)[3]
        dv[(p, b)] = d
        dv[(b, p)] = d

    nce = 0.0
    for a in range(FLIP):
        for b in range(FLIP):
            if a == b:
                N1 = 2.0 * S10[a]
                Dv = N1 + E10
                half = 10.0 - np.log(Dv) - N1 / Dv
                nce += 2.0 * half.sum()
            else:
                d = dv[(a, b)]
                N1 = S10[a] + S1[(a, b)]
                half = (
                    10.0 * d
                    - np.log(N1)
                    - 1.0
                    - np.log1p(-np.exp(10.0 * d) / N1)
                )
                nce += half.sum()

    # CE: device gives per-row sum(exp(pred)); label logit gathered on host
    se = np.concatenate(
        [np.asarray(outs[c]["m1"], np.float64)[:, 18:22].T.reshape(B) for c in range(FLIP)]
    )
    lab = np.asarray(labels).astype(np.int64)
    pred16 = np.asarray(predicts, np.float32).astype(np.float16)
    xlab = pred16[np.arange(N), lab].astype(np.float64)
    ce = (np.log(se) - xlab).mean()

    val = ALPHA * (-(nce) / 1024.0) + ce
    return np.array(val, dtype=np.float32)


def _run_hw(in_maps, trace=False):
    from concourse.bass_utils import run_bass_kernel_spmd

    nc = _get_nc()
    return run_bass_kernel_spmd(nc, in_maps, core_ids=list(range(FLIP)), trace=trace)


def kernel(predicts, labels, features, indexs=None, **_):
    in_maps = _prep_in_maps(predicts, labels, features)
    res = _run_hw(in_maps)
    return _combine(res.results, predicts, labels)


def kernel_sim(predicts, labels, features, indexs=None, **_):
    """CoreSim (CPU simulator) path for fast correctness iteration."""
    from concourse.bass_interp import CoreSim

    nc = _get_nc()
    in_maps = _prep_in_maps(predicts, labels, features)
    outs = []
    for a in range(FLIP):
        sim = CoreSim(nc, trace=False)
        for k, v in in_maps[a].items():
            sim.tensor(k)[:] = v
        sim.simulate()
        outs.append({k: np.array(sim.tensor(k)) for k in ("m1", "cs", "dv")})
    return _combine(outs, predicts, labels)


# revision 5
# speedup vs baseline: 1.9282x; 1.0058x over previous
"""Trainium2 Bass kernel for nn_BatchFlipLoss (NCE batch-flip loss + CE loss).

Restructured from the 32.5us baseline around the TRN2 cost model; ~16.9us.

Math (validated to rel-err ~1.4e-5 vs the jax reference; gate is 2e-2):
  The 36-pair NCE sum decomposes per ordered half (a,b) with
  E_ab = exp(10 G_ab), S_ab = rowsum(E_ab), d_ab[p] = f_a[p].f_b[p]:
    cross half = 10 d - ln(N1) - 1 - ln(1 - exp(10 d)/N1),  N1 = S0_aa + S_ab
    self pair  = 2*(10 - ln(D) - N1/D),  N1 = 2 S0_aa, D = N1 + e^10
  The quadratic series term (S2 = rowsum(E^2), ~6e-5 relative) is dropped.

Work split: 36 unordered blocks over 8 cores = 4.5 each. Core c owns
blocks (c, c+j) j=0..3; each distance-4 pair {p, p+4} is split by A-rows
(core p computes E rows 0:256, core p+4 rows 256:512 via host-staged lhsT).
The self block is computed full-width (complete rowsums, no colsums), so
it forms a colsum-free tail: the colsum bank and its staging copy + DMA
gate on j4's exp, ~1.7us before the exp stream ends.

Device pipeline (one SPMD program, inputs host-rotated per core):
  PE:  bf16 Gram chunk matmuls into 3 cycling 2-bank PSUM groups (p-state
       warmup matmuls run during the input DMAs); -8*I accumulated onto
       self-block diagonals; one-hot-weighted ones-matmuls accumulate the
       cross-block column sums into one zero-initialized PSUM bank.
  Act: exp(10g) fused per PSUM group, bf16 out — the only user of ScalarE
       (gap-free stream); the last group's rowsum rides the exp
       accumulator; the colsum DMA departs via ScalarE's HWDGE queue.
  DVE: per-chunk rowsums via tensor_scalar accum (bf16 4x fast mode); d
       products (own*partner); CE and the j0r2 self-block row via
       Schraudolph fast-exp (int32(A*x+B) write, bitcast-f32 read) to
       keep both off the ScalarE critical path (-8 diag shift keeps the
       affine positive in int32).
  Pool: d colsums via partition_all_reduce (partition 0 DMAd mid-stream).
Host combine: O(rows) rerouting of row/col sums between cores, closed-form
series, CE label-logit gather, final scalar.
"""

from contextlib import ExitStack

import numpy as np

FLIP = 8
B = 512
D = 128
C = 400
N = 4096
ALPHA = 0.03
E10 = float(np.exp(np.float64(10.0)))
NJ = 5

_CACHE = {}

# chunk table: (lhsT kind, lhsT idx, rhs j-slot, m1 col, cs row, rhs off, width)
# lhsT kind "own": ft[:, idx*128:(idx+1)*128]; "j4w": j4w[:, idx*128:...]
# j0 (self block) is symmetric: only the upper-triangle column slice
# [128r:512] is computed per row-chunk r; the lower-half contributions are
# reconstructed on host from the tri colsums (cst rows 8..11).
_CHUNKS = {
    # j0r0/r1 are upper-tri slices (their colsums feed r1/r2's host
    # reconstruction); r2 is a tri slice whose colsum nobody consumes
    # (csr None); r3 is computed FULL-width so it needs no reconstruction
    # at all -- r2+r3 form a colsum-free tail, letting the cst bank (and
    # its staging copy + DMA) retire ~1us before the exp stream ends.
    "j0": [("own", r, 0, r, None, 0, B) for r in range(4)],
    "j1": [("own", r, 1, 4 + r, 0, 0, B) for r in range(4)],
    "j2": [("own", r, 2, 8 + r, 1, 0, B) for r in range(4)],
    "j3": [("own", r, 3, 12 + r, 2, 0, B) for r in range(4)],
    "j4": [("j4w", c, 4, 16 + c, 3, 0, B) for c in range(2)],
}
# 11 groups of <=1024 cycling three 2-bank PSUM pools (3-deep PE->Act
# pipeline): two single-chunk groups first for the earliest exp start,
# the two colsum-free j0 slices last (the cst bank + staging copy retire
# one full group before the exp stream ends).
_GROUPS = [
    _CHUNKS["j1"][0:1],   # 512
    _CHUNKS["j1"][1:2],   # 512
    _CHUNKS["j1"][2:4],   # 1024
    _CHUNKS["j2"][0:2],   # 1024
    _CHUNKS["j2"][2:4],   # 1024
    _CHUNKS["j3"][0:2],   # 1024
    _CHUNKS["j3"][2:4],   # 1024
    _CHUNKS["j4"],        # 1024 (LAST colsum-bearing group -> early gate)
    _CHUNKS["j0"][0:2],   # 1024 (full-width: no colsums needed at all)
    _CHUNKS["j0"][3:4],   # 512 (rowsum via exp accum_out)
]
# j0 is computed FULL-width: complete rowsums need no triangle-colsum
# reconstruction, so the whole self block is colsum-free tail content and
# the cst bank (staging copy + DMA) gates on j4's exp, ~1.1us before the
# stream ends. j0r2 runs OFF the ScalarE stream via DVE Schraudolph.
_NCS = 14  # cross-block colsum matmuls only

# Schraudolph fast-exp constants for the CE path (exp(x) ~ bitcast_f32
# of int32(A*x + B)); B tuned zero-mean on the CE estimate, robust to
# trunc-vs-round int conversion (validated 6.5e-4 absolute on ce).
SCH_A = float(2**23 / np.log(2))
SCH_B = float(127 * 2**23 - 475000)


def _build_nc():
    import concourse.tile as tile
    from concourse import bacc, mybir

    f32 = mybir.dt.float32
    bf16 = mybir.dt.bfloat16
    f16 = mybir.dt.float16
    AF = mybir.ActivationFunctionType
    OP = mybir.AluOpType

    nc = bacc.Bacc("TRN2", target_bir_lowering=False, debug=False)

    ft_d = nc.dram_tensor("ft", [D, NJ * B], bf16, kind="ExternalInput")
    j4w_d = nc.dram_tensor("j4w", [D, 256], bf16, kind="ExternalInput")
    pred_d = nc.dram_tensor("pred", [128, 4 * C], f16, kind="ExternalInput")
    eye_d = nc.dram_tensor("eye2", [128, 2, 128], bf16, kind="ExternalInput")
    oh_d = nc.dram_tensor("oh", [128, 144], bf16, kind="ExternalInput")
    m1_d = nc.dram_tensor("m1", [128, 22], f32, kind="ExternalOutput")
    cs_d = nc.dram_tensor("cs", [12, B], f32, kind="ExternalOutput")
    d_d = nc.dram_tensor("dv", [4, B], f32, kind="ExternalOutput")

    with tile.TileContext(nc) as tc, ExitStack() as ctx:
        const = ctx.enter_context(tc.tile_pool(name="const", bufs=1))
        pg = [
            ctx.enter_context(tc.tile_pool(name=f"pg{i}", bufs=1, space="PSUM"))
            for i in range(3)
        ]
        pwu = ctx.enter_context(tc.tile_pool(name="pwu", bufs=1, space="PSUM"))
        pcs = ctx.enter_context(tc.tile_pool(name="pcs", bufs=1, space="PSUM"))
        pet = ctx.enter_context(tc.tile_pool(name="pet", bufs=5))
        pscr = ctx.enter_context(tc.tile_pool(name="pscr", bufs=2))
        small = ctx.enter_context(tc.tile_pool(name="small", bufs=1))

        ftt = const.tile([D, NJ * B], bf16)
        j4wt = const.tile([D, 256], bf16)
        predt = const.tile([128, 4 * C], f16)
        eyet = const.tile([128, 2, 128], bf16)
        oht = const.tile([128, 144], bf16)
        M1 = small.tile([128, 22], f32)
        cs_s = small.tile([12, B], f32)
        ce_i32 = small.tile([128, 4 * C], mybir.dt.int32)
        db = small.tile([128, 4, B], bf16)
        dred = small.tile([128, 4, B], f32)
        sj32 = small.tile([128, B], mybir.dt.int32)
        sjbf = small.tile([128, B], bf16)
        wt2h = [None]

        # input DMAs in Gram-pipeline priority order: own+j1 block first
        # (unblocks fills 0-2), then j2, then j3+j4rhs, then the rest —
        # large DMAs occupy all engines sequentially, so order is latency.
        nc.sync.dma_start(ftt[:, 0 : 2 * B], ft_d[:, 0 : 2 * B])
        nc.sync.dma_start(oht[:], oh_d[:, :])  # tiny; colsums need it early
        nc.sync.dma_start(ftt[:, 2 * B : 3 * B], ft_d[:, 2 * B : 3 * B])
        nc.sync.dma_start(ftt[:, 3 * B :], ft_d[:, 3 * B :])
        nc.sync.dma_start(eyet[:], eye_d[:, :])
        nc.sync.dma_start(j4wt[:], j4w_d[:, :])
        nc.sync.dma_start(predt[:], pred_d[:, :])

        # colsum accumulator bank: rows 0-2 cs j1-3, 3 cs j4, 4-7 d j1-4,
        # 8-10 j0 tri colsums (row 8+r holds block-cols 128r.. at offset 0).
        # Zero-initialized so every colsum matmul can accumulate with
        # start=False — the scheduler may reorder accumulating matmuls, so
        # no single one can safely carry the start flag.
        cst = pcs.tile([12, B], f32)
        nc.vector.memset(cst[:], 0.0)

        # ---- PE p-state warmup: dummy matmuls on a memset tile while the
        # input DMAs land, so real matmuls start at full clock (the Tensor
        # engine needs ~3us of continuous execution to leave mid p-state).
        # Dedicated PSUM bank so no WAW dependency delays the real fills.
        wu = const.tile([128, B], bf16)
        nc.gpsimd.memset(wu[:], 0.0625)
        warm = pwu.tile([128, B], f32, tag="wu")
        for i in range(5):
            nc.tensor.matmul(
                warm[:, :],
                wu[:, 0:128],
                wu[:],
                start=True,
                stop=True,
                skip_group_check=True,
            )



        # ---- Gram pipeline ----
        ngroups = len(_GROUPS)
        ets = [None] * ngroups
        gts = [None] * ngroups
        spans = [None] * ngroups

        def _offsets(chunks):
            offs, o = [], 0
            for ch in chunks:
                offs.append(o)
                o += ch[6]
            return offs, o

        def fill_group(gi):
            chunks = _GROUPS[gi]
            offs, w = _offsets(chunks)
            pool = pg[gi % 3]
            gt = pool.tile([128, 1024], f32, tag=f"g{gi % 3}")
            for (kind, idx, j, m1c, csr, roff, width), o in zip(chunks, offs):
                lhsT = (
                    ftt[:, idx * 128 : (idx + 1) * 128]
                    if kind == "own"
                    else j4wt[:, idx * 128 : (idx + 1) * 128]
                )
                nc.tensor.matmul(
                    gt[:, o : o + width],
                    lhsT,
                    ftt[:, j * B + roff : j * B + roff + width],
                    start=True,
                    stop=(j != 0),
                    skip_group_check=(j == 0),
                )
                if j == 0:
                    # own-block diag: accumulate -8*I; exp(10(g-8)) ~ 4e-31
                    # (negligible in the sums; -8 keeps the Schraudolph
                    # affine for the DVE j0r2 path positive in int32)
                    dg = o + idx * 128 - roff
                    nc.tensor.matmul(
                        gt[:, dg : dg + 128],
                        eyet[:, 0, :],
                        eyet[:, 1, :],
                        start=False,
                        stop=True,
                        skip_group_check=True,
                    )
            gts[gi] = gt
            spans[gi] = w

        def exp_group(gi, accum_m1c=None):
            w = spans[gi]
            et = pet.tile([128, 1024], bf16, tag="et")
            kw = {}
            if accum_m1c is not None:
                # last group: the rowsum rides the exp's own accumulator
                # (+187ns on ScalarE) instead of a DVE pass that would race
                # the cst staging copy at the tail
                kw["accum_out"] = M1[:, accum_m1c : accum_m1c + 1]
            nc.scalar.activation(
                et[:, 0:w], gts[gi][:, 0:w], AF.Exp, bias=0.0, scale=10.0, **kw
            )
            ets[gi] = et

        def sums_group(gi):
            chunks = _GROUPS[gi]
            offs, _ = _offsets(chunks)
            et = ets[gi]
            for (kind, idx, j, m1c, csr, roff, width), o in zip(chunks, offs):
                scr = pscr.tile([128, B], bf16, tag="scr")
                nc.vector.tensor_scalar(
                    scr[:, 0:width],
                    et[:, o : o + width],
                    1.0,
                    None,
                    OP.mult,
                    OP.add,
                    accum_out=M1[:, m1c : m1c + 1],
                )

        # all colsum matmuls form ONE accumulation group into cst [8,512]:
        # lhsT = one-hot column csr of ones -> adds rowsum into row csr
        NCS = _NCS
        cs_count = [0]

        def cs_matmul(csr, rhs, width=B):
            i = cs_count[0]
            cs_count[0] += 1
            nc.tensor.matmul(
                cst[:, 0:width],
                oht[:, csr * 12 : (csr + 1) * 12],
                rhs,
                start=False,
                stop=(i == NCS - 1),
                skip_group_check=True,
            )

        def cs_group(gi):
            chunks = _GROUPS[gi]
            offs, _ = _offsets(chunks)
            et = ets[gi]
            for (kind, idx, j, m1c, csr, roff, width), o in zip(chunks, offs):
                if csr is None:
                    continue
                cs_matmul(csr, et[:, o : o + width], width)

        # PE order: g0, g1, d-colsums, then fill g(i+1) before cs(g i-1)
        fill_group(0)
        exp_group(0)
        fill_group(1)
        exp_group(1)
        fill_group(2)
        exp_group(2)
        sums_group(0)
        sums_group(1)
        for gi in range(3, ngroups):
            fill_group(gi)
            if gi == ngroups - 1:
                exp_group(gi, accum_m1c=_GROUPS[gi][0][3])
            else:
                exp_group(gi)
            cs_group(gi - 3)
            sums_group(gi - 1)  # (sums 0,1 issued above)
            # DVE filler work goes after the pipeline-critical sums so the
            # scheduler always prefers sums (they gate et-slot recycling)
            if gi == 3:
                # d products (elementwise own*partner on DVE); the column
                # sums run on the otherwise-idle GPSIMD engine (result is
                # partition-broadcast, partition 0 is DMAd out mid-stream)
                # so neither PE nor the cst staging copy is involved.
                ftv = ftt[:].rearrange("p (j b) -> p j b", j=NJ)
                nc.vector.tensor_tensor(
                    db[:],
                    ftv[:, 1:NJ, :],
                    ftv[:, 0:1, :].to_broadcast([128, 4, B]),
                    OP.mult,
                )
                from concourse import bass_isa

                nc.gpsimd.partition_all_reduce(
                    dred[:], db[:], 128, bass_isa.ReduceOp.add
                )
                nc.sync.dma_start(d_d[:, :], dred[0:1, :, :])
            elif gi == 4:
                # CE on DVE via Schraudolph fast-exp: int32(A*x+B) then
                # bitcast-f32 rowsums; frees ScalarE for the Gram exps.
                nc.vector.tensor_scalar(
                    ce_i32[:], predt[:], SCH_A, SCH_B, OP.mult, OP.add
                )
            elif gi == 5:
                ce_f32 = ce_i32[:].bitcast(f32)
                for c in range(4):
                    scr2 = pscr.tile([128, B], f32, tag="scr2")
                    nc.vector.tensor_scalar(
                        scr2[:, 0:C],
                        ce_f32[:, c * C : (c + 1) * C],
                        1.0,
                        None,
                        OP.mult,
                        OP.add,
                        accum_out=M1[:, 18 + c : 19 + c],
                    )
            elif gi == 6:
                # j0r2 Gram (full row) into the spare warmup bank
                wt2 = pwu.tile([128, B], f32, tag="wu")
                wt2h[0] = wt2
                nc.tensor.matmul(
                    wt2[:, :],
                    ftt[:, 256:384],
                    ftt[:, 0:B],
                    start=True,
                    stop=False,
                    skip_group_check=True,
                )
                nc.tensor.matmul(
                    wt2[:, 256:384],
                    eyet[:, 0, :],
                    eyet[:, 1, :],
                    start=False,
                    stop=True,
                    skip_group_check=True,
                )
            elif gi == 7:
                # j0r2 exp via DVE Schraudolph: int32(10A*g + B), bitcast
                # f32 -> bf16 with the rowsum riding the accum (-8-shifted
                # diag keeps the affine positive; residual ~2^-101)
                nc.vector.tensor_scalar(
                    sj32[:],
                    wt2h[0][:, :],
                    10.0 * SCH_A,
                    SCH_B,
                    OP.mult,
                    OP.add,
                )
                nc.vector.tensor_scalar(
                    sjbf[:],
                    sj32[:].bitcast(f32),
                    1.0,
                    None,
                    OP.mult,
                    OP.add,
                    accum_out=M1[:, 2:3],
                )
        cs_group(ngroups - 3)  # j4 colsums
        nc.sync.dma_start(m1_d[:, :], M1[:])
        # stage colsum bank to SBUF, then DMA out via ScalarE's HWDGE
        # queue (idle at the tail) so the terminal m1 DMA has the SP queue
        # to itself
        nc.vector.tensor_copy(cs_s[:], cst[:])
        nc.scalar.dma_start(cs_d[:, :], cs_s[:])

    nc.compile()
    return nc


def _get_nc():
    if "nc" not in _CACHE:
        _CACHE["nc"] = _build_nc()
    return _CACHE["nc"]


def _prep_in_maps(predicts, labels, features):
    import ml_dtypes

    feats = np.ascontiguousarray(features, dtype=np.float32)
    pred = np.ascontiguousarray(predicts, dtype=np.float32)
    f8 = feats.reshape(B, FLIP, D).transpose(1, 0, 2)  # [8,512,128]
    eye2 = np.stack(
        [-8.0 * np.eye(128, dtype=np.float32), np.eye(128, dtype=np.float32)], axis=1
    ).astype(ml_dtypes.bfloat16)  # [128, 2, 128]: lhsT=-8I, rhs=I
    oh = np.zeros((128, 12, 12), dtype=np.float32)
    for r in range(12):
        oh[:, r, r] = 1.0
    oh = oh.reshape(128, 144).astype(ml_dtypes.bfloat16)
    in_maps = []
    for a in range(FLIP):
        order = [(a + i) % FLIP for i in range(NJ)]
        fo = f8[order].copy()  # [5, 512, 128]
        if a >= 4:
            fo[4] = f8[a]  # j4 Gram rhs = own (pair-B side)
        ft = np.ascontiguousarray(fo.transpose(2, 0, 1).reshape(D, NJ * B)).astype(
            ml_dtypes.bfloat16
        )
        pa = a if a < 4 else a - 4
        rows = slice(0, 256) if a < 4 else slice(256, 512)
        j4w = np.ascontiguousarray(f8[pa][rows].T).astype(ml_dtypes.bfloat16)
        pr = (
            pred[a * B : (a + 1) * B]
            .reshape(4, 128, C)
            .transpose(1, 0, 2)
            .reshape(128, 4 * C)
        )
        in_maps.append(
            {
                "ft": ft,
                "j4w": np.ascontiguousarray(j4w),
                "pred": np.ascontiguousarray(pr).astype(np.float16),
                "eye2": np.ascontiguousarray(eye2),
                "oh": oh,
            }
        )
    return in_maps


def _combine(outs, predicts, labels):
    """Host O(rows) combine: reroute per-block sums, closed-form series."""
    S1 = {}
    S10 = {}
    dv = {}
    for c in range(FLIP):
        m1 = np.asarray(outs[c]["m1"], np.float64)  # [128, 22]
        cs = np.asarray(outs[c]["cs"], np.float64)  # [12, 512]
        dvr = np.asarray(outs[c]["dv"], np.float64)  # [4, 512]
        S10[c] = m1[:, 0:4].T.reshape(B)  # full-width diag-zeroed rowsums
        for j in (1, 2, 3):
            b = (c + j) % FLIP
            S1[(c, b)] = m1[:, j * 4 : (j + 1) * 4].T.reshape(B)
            S1[(b, c)] = cs[j - 1]
            dv[(c, b)] = dvr[j - 1]
            dv[(b, c)] = dvr[j - 1]
    for p in range(4):
        b = p + 4
        S1[(p, b)] = np.concatenate(
            [
                np.asarray(outs[p]["m1"], np.float64)[:, 16:18].T.reshape(256),
                np.asarray(outs[b]["m1"], np.float64)[:, 16:18].T.reshape(256),
            ]
        )
        S1[(b, p)] = (
            np.asarray(outs[p]["cs"], np.float64)[3]
            + np.asarray(outs[b]["cs"], np.float64)[3]
        )
        d = np.asarray(outs[p]["dv"], np.float64)[3]
        dv[(p, b)] = d
        dv[(b, p)] = d

    nce = 0.0
    for a in range(FLIP):
        for b in range(FLIP):
            if a == b:
                N1 = 2.0 * S10[a]
                Dv = N1 + E10
                half = 10.0 - np.log(Dv) - N1 / Dv
                nce += 2.0 * half.sum()
            else:
                d = dv[(a, b)]
                N1 = S10[a] + S1[(a, b)]
                half = (
                    10.0 * d
                    - np.log(N1)
                    - 1.0
                    - np.log1p(-np.exp(10.0 * d) / N1)
                )
                nce += half.sum()

    # CE: device gives per-row sum(exp(pred)); label logit gathered on host
    se = np.concatenate(
        [np.asarray(outs[c]["m1"], np.float64)[:, 18:22].T.reshape(B) for c in range(FLIP)]
    )
    lab = np.asarray(labels).astype(np.int64)
    pred16 = np.asarray(predicts, np.float32).astype(np.float16)
    xlab = pred16[np.arange(N), lab].astype(np.float64)
    ce = (np.log(se) - xlab).mean()

    val = ALPHA * (-(nce) / 1024.0) + ce
    return np.array(val, dtype=np.float32)


def _run_hw(in_maps, trace=False):
    from concourse.bass_utils import run_bass_kernel_spmd

    nc = _get_nc()
    return run_bass_kernel_spmd(nc, in_maps, core_ids=list(range(FLIP)), trace=trace)


def kernel(predicts, labels, features, indexs=None, **_):
    in_maps = _prep_in_maps(predicts, labels, features)
    res = _run_hw(in_maps)
    return _combine(res.results, predicts, labels)


def kernel_sim(predicts, labels, features, indexs=None, **_):
    """CoreSim (CPU simulator) path for fast correctness iteration."""
    from concourse.bass_interp import CoreSim

    nc = _get_nc()
    in_maps = _prep_in_maps(predicts, labels, features)
    outs = []
    for a in range(FLIP):
        sim = CoreSim(nc, trace=False)
        for k, v in in_maps[a].items():
            sim.tensor(k)[:] = v
        sim.simulate()
        outs.append({k: np.array(sim.tensor(k)) for k in ("m1", "cs", "dv")})
    return _combine(outs, predicts, labels)


# revision 6
# speedup vs baseline: 1.9559x; 1.0144x over previous
"""Trainium2 Bass kernel for nn_BatchFlipLoss (NCE batch-flip loss + CE loss).

Restructured from the 32.5us baseline around the TRN2 cost model; ~16.6us.

Math (validated to rel-err ~1.4e-5 vs the jax reference; gate is 2e-2):
  The 36-pair NCE sum decomposes per ordered half (a,b) with
  E_ab = exp(10 G_ab), S_ab = rowsum(E_ab), d_ab[p] = f_a[p].f_b[p]:
    cross half = 10 d - ln(N1) - 1 - ln(1 - exp(10 d)/N1),  N1 = S0_aa + S_ab
    self pair  = 2*(10 - ln(D) - N1/D),  N1 = 2 S0_aa, D = N1 + e^10
  The quadratic series term (S2 = rowsum(E^2), ~6e-5 relative) is dropped.

Work split: 36 unordered blocks over 8 cores = 4.5 each. Core c owns
blocks (c, c+j) j=0..3; each distance-4 pair {p, p+4} is split by A-rows
(core p computes E rows 0:256, core p+4 rows 256:512 via host-staged lhsT).
The self block is computed full-width (complete rowsums, no colsums), so
it forms a colsum-free tail: the colsum bank and its staging copy + DMA
gate on j4's exp, well before the exp stream ends. The ft column layout
is [j1 | own | j2 | j3 | j4rhs] so the first DMA piece (j1 + own r0,
cols 0:640) is minimal and the first matmul starts ~3.3us in.

Device pipeline (one SPMD program, inputs host-rotated per core):
  PE:  bf16 Gram chunk matmuls into 3 cycling 2-bank PSUM groups (p-state
       warmup matmuls run during the input DMAs); -8*I accumulated onto
       self-block diagonals; one-hot-weighted ones-matmuls accumulate the
       cross-block column sums into one zero-initialized PSUM bank.
  Act: exp(10g) fused per PSUM group, bf16 out — the only user of ScalarE
       (gap-free stream); the last group's rowsum rides the exp
       accumulator; the colsum DMA departs via ScalarE's HWDGE queue.
  DVE: per-chunk rowsums via tensor_scalar accum (bf16 4x fast mode); d
       products (own*partner); CE and the j0r2 self-block row via
       Schraudolph fast-exp (int32(A*x+B) write, bitcast-f32 read) to
       keep both off the ScalarE critical path (-8 diag shift keeps the
       affine positive in int32).
  Pool: d colsums via partition_all_reduce (partition 0 DMAd mid-stream).
Host combine: O(rows) rerouting of row/col sums between cores, closed-form
series, CE label-logit gather, final scalar.
"""

from contextlib import ExitStack

import numpy as np

FLIP = 8
B = 512
D = 128
C = 400
N = 4096
ALPHA = 0.03
E10 = float(np.exp(np.float64(10.0)))
NJ = 5

_CACHE = {}

# ft column layout [j1 | own | j2 | j3 | j4rhs]: the first DMA piece
# (cols 0:640 = j1 rhs + own r0 lhsT) is minimal -> earliest first matmul.
_JOFF = {0: 512, 1: 0, 2: 1024, 3: 1536, 4: 2048}
_OWN = 512
# chunk table: (lhsT kind, lhsT idx, rhs j-slot, m1 col, cs row, rhs off, width)
# lhsT kind "own": ft[:, idx*128:(idx+1)*128]; "j4w": j4w[:, idx*128:...]
# j0 (self block) is symmetric: only the upper-triangle column slice
# [128r:512] is computed per row-chunk r; the lower-half contributions are
# reconstructed on host from the tri colsums (cst rows 8..11).
_CHUNKS = {
    # j0r0/r1 are upper-tri slices (their colsums feed r1/r2's host
    # reconstruction); r2 is a tri slice whose colsum nobody consumes
    # (csr None); r3 is computed FULL-width so it needs no reconstruction
    # at all -- r2+r3 form a colsum-free tail, letting the cst bank (and
    # its staging copy + DMA) retire ~1us before the exp stream ends.
    "j0": [("own", r, 0, r, None, 0, B) for r in range(4)],
    "j1": [("own", r, 1, 4 + r, 0, 0, B) for r in range(4)],
    "j2": [("own", r, 2, 8 + r, 1, 0, B) for r in range(4)],
    "j3": [("own", r, 3, 12 + r, 2, 0, B) for r in range(4)],
    "j4": [("j4w", c, 4, 16 + c, 3, 0, B) for c in range(2)],
}
# 11 groups of <=1024 cycling three 2-bank PSUM pools (3-deep PE->Act
# pipeline): two single-chunk groups first for the earliest exp start,
# the two colsum-free j0 slices last (the cst bank + staging copy retire
# one full group before the exp stream ends).
_GROUPS = [
    _CHUNKS["j1"][0:1],   # 512
    _CHUNKS["j1"][1:2],   # 512
    _CHUNKS["j1"][2:4],   # 1024
    _CHUNKS["j2"][0:2],   # 1024
    _CHUNKS["j2"][2:4],   # 1024
    _CHUNKS["j3"][0:2],   # 1024
    _CHUNKS["j3"][2:4],   # 1024
    _CHUNKS["j4"],        # 1024 (LAST colsum-bearing group -> early gate)
    _CHUNKS["j0"][0:2],   # 1024 (full-width: no colsums needed)
    _CHUNKS["j0"][3:4],   # 512 (rowsum via exp accum_out)
]
# j0 is computed FULL-width: complete rowsums need no triangle-colsum
# reconstruction, so the whole self block is colsum-free tail content and
# the cst bank (staging copy + DMA) gates on j4's exp, ~1.7us before the
# stream ends. j0r1 AND j0r2 run OFF the ScalarE stream via DVE
# Schraudolph (serialized through the spare PSUM bank); the CE rowsums
# move to the otherwise-idle GPSIMD to free the DVE budget for them.
_NCS = 14  # cross-block colsum matmuls only

# Schraudolph fast-exp constants for the CE path (exp(x) ~ bitcast_f32
# of int32(A*x + B)); B tuned zero-mean on the CE estimate, robust to
# trunc-vs-round int conversion (validated 6.5e-4 absolute on ce).
SCH_A = float(2**23 / np.log(2))
SCH_B = float(127 * 2**23 - 475000)


def _build_nc():
    import concourse.tile as tile
    from concourse import bacc, mybir

    f32 = mybir.dt.float32
    bf16 = mybir.dt.bfloat16
    f16 = mybir.dt.float16
    AF = mybir.ActivationFunctionType
    OP = mybir.AluOpType

    nc = bacc.Bacc("TRN2", target_bir_lowering=False, debug=False)

    ft_d = nc.dram_tensor("ft", [D, NJ * B], bf16, kind="ExternalInput")
    j4w_d = nc.dram_tensor("j4w", [D, 256], bf16, kind="ExternalInput")
    pred_d = nc.dram_tensor("pred", [128, 4 * C], f16, kind="ExternalInput")
    eye_d = nc.dram_tensor("eye2", [128, 2, 128], bf16, kind="ExternalInput")
    oh_d = nc.dram_tensor("oh", [128, 144], bf16, kind="ExternalInput")
    m1_d = nc.dram_tensor("m1", [128, 22], f32, kind="ExternalOutput")
    cs_d = nc.dram_tensor("cs", [12, B], f32, kind="ExternalOutput")
    d_d = nc.dram_tensor("dv", [4, B], f32, kind="ExternalOutput")

    with tile.TileContext(nc) as tc, ExitStack() as ctx:
        const = ctx.enter_context(tc.tile_pool(name="const", bufs=1))
        pg = [
            ctx.enter_context(tc.tile_pool(name=f"pg{i}", bufs=1, space="PSUM"))
            for i in range(3)
        ]
        pwu = ctx.enter_context(tc.tile_pool(name="pwu", bufs=1, space="PSUM"))
        pcs = ctx.enter_context(tc.tile_pool(name="pcs", bufs=1, space="PSUM"))
        pet = ctx.enter_context(tc.tile_pool(name="pet", bufs=5))
        pscr = ctx.enter_context(tc.tile_pool(name="pscr", bufs=2))
        small = ctx.enter_context(tc.tile_pool(name="small", bufs=1))

        ftt = const.tile([D, NJ * B], bf16)
        j4wt = const.tile([D, 256], bf16)
        predt = const.tile([128, 4 * C], f16)
        eyet = const.tile([128, 2, 128], bf16)
        oht = const.tile([128, 144], bf16)
        M1 = small.tile([128, 22], f32)
        cs_s = small.tile([12, B], f32)
        ce_i32 = small.tile([128, 4 * C], mybir.dt.int32)
        db = small.tile([128, 4, B], bf16)
        dred = small.tile([128, 4, B], f32)
        sj32 = small.tile([128, B], mybir.dt.int32)
        sjbf = small.tile([128, B], bf16)
        wt2h = [None]
        wt3h = [None]

        # input DMAs in Gram-pipeline priority order: own+j1 block first
        # (unblocks fills 0-2), then j2, then j3+j4rhs, then the rest —
        # large DMAs occupy all engines sequentially, so order is latency.
        nc.sync.dma_start(ftt[:, 0:640], ft_d[:, 0:640])  # j1 + own r0
        nc.sync.dma_start(ftt[:, 640:1536], ft_d[:, 640:1536])  # own r1-3 + j2
        nc.sync.dma_start(oht[:], oh_d[:, :])  # tiny; colsums need it early
        nc.sync.dma_start(ftt[:, 1536:], ft_d[:, 1536:])  # j3 + j4rhs
        nc.sync.dma_start(eyet[:], eye_d[:, :])
        nc.sync.dma_start(j4wt[:], j4w_d[:, :])
        nc.sync.dma_start(predt[:], pred_d[:, :])

        # colsum accumulator bank: rows 0-2 cs j1-3, 3 cs j4, 4-7 d j1-4,
        # 8-10 j0 tri colsums (row 8+r holds block-cols 128r.. at offset 0).
        # Zero-initialized so every colsum matmul can accumulate with
        # start=False — the scheduler may reorder accumulating matmuls, so
        # no single one can safely carry the start flag.
        cst = pcs.tile([12, B], f32)
        nc.vector.memset(cst[:], 0.0)

        # ---- PE p-state warmup: dummy matmuls on a memset tile while the
        # input DMAs land, so real matmuls start at full clock (the Tensor
        # engine needs ~3us of continuous execution to leave mid p-state).
        # Dedicated PSUM bank so no WAW dependency delays the real fills.
        wu = const.tile([128, B], bf16)
        nc.gpsimd.memset(wu[:], 0.0625)
        warm = pwu.tile([128, B], f32, tag="wu")
        for i in range(5):
            nc.tensor.matmul(
                warm[:, :],
                wu[:, 0:128],
                wu[:],
                start=True,
                stop=True,
                skip_group_check=True,
            )



        # ---- Gram pipeline ----
        ngroups = len(_GROUPS)
        ets = [None] * ngroups
        gts = [None] * ngroups
        spans = [None] * ngroups

        def _offsets(chunks):
            offs, o = [], 0
            for ch in chunks:
                offs.append(o)
                o += ch[6]
            return offs, o

        def fill_group(gi):
            chunks = _GROUPS[gi]
            offs, w = _offsets(chunks)
            pool = pg[gi % 3]
            gt = pool.tile([128, 1024], f32, tag=f"g{gi % 3}")
            for (kind, idx, j, m1c, csr, roff, width), o in zip(chunks, offs):
                lhsT = (
                    ftt[:, _OWN + idx * 128 : _OWN + (idx + 1) * 128]
                    if kind == "own"
                    else j4wt[:, idx * 128 : (idx + 1) * 128]
                )
                nc.tensor.matmul(
                    gt[:, o : o + width],
                    lhsT,
                    ftt[:, _JOFF[j] + roff : _JOFF[j] + roff + width],
                    start=True,
                    stop=(j != 0),
                    skip_group_check=(j == 0),
                )
                if j == 0:
                    # own-block diag: accumulate -8*I; exp(10(g-8)) ~ 4e-31
                    # (negligible in the sums; -8 keeps the Schraudolph
                    # affine for the DVE j0r2 path positive in int32)
                    dg = o + idx * 128 - roff
                    nc.tensor.matmul(
                        gt[:, dg : dg + 128],
                        eyet[:, 0, :],
                        eyet[:, 1, :],
                        start=False,
                        stop=True,
                        skip_group_check=True,
                    )
            gts[gi] = gt
            spans[gi] = w

        def exp_group(gi, accum_m1c=None):
            w = spans[gi]
            et = pet.tile([128, 1024], bf16, tag="et")
            kw = {}
            if accum_m1c is not None:
                # last group: the rowsum rides the exp's own accumulator
                # (+187ns on ScalarE) instead of a DVE pass that would race
                # the cst staging copy at the tail
                kw["accum_out"] = M1[:, accum_m1c : accum_m1c + 1]
            nc.scalar.activation(
                et[:, 0:w], gts[gi][:, 0:w], AF.Exp, bias=0.0, scale=10.0, **kw
            )
            ets[gi] = et

        def sums_group(gi):
            chunks = _GROUPS[gi]
            offs, _ = _offsets(chunks)
            et = ets[gi]
            for (kind, idx, j, m1c, csr, roff, width), o in zip(chunks, offs):
                scr = pscr.tile([128, B], bf16, tag="scr")
                nc.vector.tensor_scalar(
                    scr[:, 0:width],
                    et[:, o : o + width],
                    1.0,
                    None,
                    OP.mult,
                    OP.add,
                    accum_out=M1[:, m1c : m1c + 1],
                )

        # all colsum matmuls form ONE accumulation group into cst [8,512]:
        # lhsT = one-hot column csr of ones -> adds rowsum into row csr
        NCS = _NCS
        cs_count = [0]

        def cs_matmul(csr, rhs, width=B):
            i = cs_count[0]
            cs_count[0] += 1
            nc.tensor.matmul(
                cst[:, 0:width],
                oht[:, csr * 12 : (csr + 1) * 12],
                rhs,
                start=False,
                stop=(i == NCS - 1),
                skip_group_check=True,
            )

        def cs_group(gi):
            chunks = _GROUPS[gi]
            offs, _ = _offsets(chunks)
            et = ets[gi]
            for (kind, idx, j, m1c, csr, roff, width), o in zip(chunks, offs):
                if csr is None:
                    continue
                cs_matmul(csr, et[:, o : o + width], width)

        # PE order: g0, g1, d-colsums, then fill g(i+1) before cs(g i-1)
        fill_group(0)
        exp_group(0)
        fill_group(1)
        exp_group(1)
        fill_group(2)
        exp_group(2)
        sums_group(0)
        sums_group(1)
        for gi in range(3, ngroups):
            fill_group(gi)
            if gi == ngroups - 1:
                exp_group(gi, accum_m1c=_GROUPS[gi][0][3])
            else:
                exp_group(gi)
            cs_group(gi - 3)
            sums_group(gi - 1)  # (sums 0,1 issued above)
            # DVE filler work goes after the pipeline-critical sums so the
            # scheduler always prefers sums (they gate et-slot recycling)
            if gi == 3:
                # d products (elementwise own*partner on DVE); the column
                # sums run on the otherwise-idle GPSIMD engine (result is
                # partition-broadcast, partition 0 is DMAd out mid-stream)
                # so neither PE nor the cst staging copy is involved.
                ftv = ftt[:].rearrange("p (j b) -> p j b", j=NJ)
                own_b = ftv[:, 1:2, :]
                nc.vector.tensor_tensor(
                    db[:, 0:1, :], ftv[:, 0:1, :], own_b, OP.mult
                )
                nc.vector.tensor_tensor(
                    db[:, 1:4, :],
                    ftv[:, 2:NJ, :],
                    own_b.to_broadcast([128, 3, B]),
                    OP.mult,
                )
                from concourse import bass_isa

                nc.gpsimd.partition_all_reduce(
                    dred[:], db[:], 128, bass_isa.ReduceOp.add
                )
                nc.sync.dma_start(d_d[:, :], dred[0:1, :, :])
            elif gi == 4:
                # CE on DVE via Schraudolph fast-exp: int32(A*x+B) then
                # bitcast-f32 rowsums; frees ScalarE for the Gram exps.
                nc.vector.tensor_scalar(
                    ce_i32[:], predt[:], SCH_A, SCH_B, OP.mult, OP.add
                )
            elif gi == 5:
                # CE rowsums (DVE; walrus rejects TensorScalarPtr on Pool)
                ce_f32 = ce_i32[:].bitcast(f32)
                for c in range(4):
                    scr2 = pscr.tile([128, B], f32, tag="scr2")
                    nc.vector.tensor_scalar(
                        scr2[:, 0:C],
                        ce_f32[:, c * C : (c + 1) * C],
                        1.0,
                        None,
                        OP.mult,
                        OP.add,
                        accum_out=M1[:, 18 + c : 19 + c],
                    )
            elif gi == 6:
                # j0r2 Gram (full row) into the spare warmup bank
                wt2 = pwu.tile([128, B], f32, tag="wu")
                wt2h[0] = wt2
                nc.tensor.matmul(
                    wt2[:, :],
                    ftt[:, _OWN + 256 : _OWN + 384],
                    ftt[:, _OWN : _OWN + B],
                    start=True,
                    stop=False,
                    skip_group_check=True,
                )
                nc.tensor.matmul(
                    wt2[:, 256:384],
                    eyet[:, 0, :],
                    eyet[:, 1, :],
                    start=False,
                    stop=True,
                    skip_group_check=True,
                )
            elif gi == 7:
                # j0r2 exp via DVE Schraudolph: int32(10A*g + B), bitcast
                # f32 -> bf16 with the rowsum riding the accum (-8-shifted
                # diag keeps the affine positive; residual ~2^-101)
                nc.vector.tensor_scalar(
                    sj32[:],
                    wt2h[0][:, :],
                    10.0 * SCH_A,
                    SCH_B,
                    OP.mult,
                    OP.add,
                )
                nc.vector.tensor_scalar(
                    sjbf[:],
                    sj32[:].bitcast(f32),
                    1.0,
                    None,
                    OP.mult,
                    OP.add,
                    accum_out=M1[:, 2:3],
                )
        cs_group(ngroups - 3)  # j4 colsums
        nc.sync.dma_start(m1_d[:, :], M1[:])
        # stage colsum bank to SBUF, then DMA out via ScalarE's HWDGE
        # queue (idle at the tail) so the terminal m1 DMA has the SP queue
        # to itself
        nc.vector.tensor_copy(cs_s[:], cst[:])
        nc.scalar.dma_start(cs_d[:, :], cs_s[:])

    nc.compile()
    return nc


def _get_nc():
    if "nc" not in _CACHE:
        _CACHE["nc"] = _build_nc()
    return _CACHE["nc"]


def _prep_in_maps(predicts, labels, features):
    import ml_dtypes

    feats = np.ascontiguousarray(features, dtype=np.float32)
    pred = np.ascontiguousarray(predicts, dtype=np.float32)
    f8 = feats.reshape(B, FLIP, D).transpose(1, 0, 2)  # [8,512,128]
    eye2 = np.stack(
        [-8.0 * np.eye(128, dtype=np.float32), np.eye(128, dtype=np.float32)], axis=1
    ).astype(ml_dtypes.bfloat16)  # [128, 2, 128]: lhsT=-8I, rhs=I
    oh = np.zeros((128, 12, 12), dtype=np.float32)
    for r in range(12):
        oh[:, r, r] = 1.0
    oh = oh.reshape(128, 144).astype(ml_dtypes.bfloat16)
    in_maps = []
    for a in range(FLIP):
        order = [(a + 1) % FLIP, a, (a + 2) % FLIP, (a + 3) % FLIP, (a + 4) % FLIP]
        fo = f8[order].copy()  # [5, 512, 128]: [j1 | own | j2 | j3 | j4rhs]
        if a >= 4:
            fo[4] = f8[a]  # j4 Gram rhs = own (pair-B side)
        ft = np.ascontiguousarray(fo.transpose(2, 0, 1).reshape(D, NJ * B)).astype(
            ml_dtypes.bfloat16
        )
        pa = a if a < 4 else a - 4
        rows = slice(0, 256) if a < 4 else slice(256, 512)
        j4w = np.ascontiguousarray(f8[pa][rows].T).astype(ml_dtypes.bfloat16)
        pr = (
            pred[a * B : (a + 1) * B]
            .reshape(4, 128, C)
            .transpose(1, 0, 2)
            .reshape(128, 4 * C)
        )
        in_maps.append(
            {
                "ft": ft,
                "j4w": np.ascontiguousarray(j4w),
                "pred": np.ascontiguousarray(pr).astype(np.float16),
                "eye2": np.ascontiguousarray(eye2),
                "oh": oh,
            }
        )
    return in_maps


def _combine(outs, predicts, labels):
    """Host O(rows) combine: reroute per-block sums, closed-form series."""
    S1 = {}
    S10 = {}
    dv = {}
    for c in range(FLIP):
        m1 = np.asarray(outs[c]["m1"], np.float64)  # [128, 22]
        cs = np.asarray(outs[c]["cs"], np.float64)  # [12, 512]
        dvr = np.asarray(outs[c]["dv"], np.float64)  # [4, 512]
        S10[c] = m1[:, 0:4].T.reshape(B)  # full-width diag-zeroed rowsums
        for j in (1, 2, 3):
            b = (c + j) % FLIP
            S1[(c, b)] = m1[:, j * 4 : (j + 1) * 4].T.reshape(B)
            S1[(b, c)] = cs[j - 1]
            dv[(c, b)] = dvr[j - 1]
            dv[(b, c)] = dvr[j - 1]
    for p in range(4):
        b = p + 4
        S1[(p, b)] = np.concatenate(
            [
                np.asarray(outs[p]["m1"], np.float64)[:, 16:18].T.reshape(256),
                np.asarray(outs[b]["m1"], np.float64)[:, 16:18].T.reshape(256),
            ]
        )
        S1[(b, p)] = (
            np.asarray(outs[p]["cs"], np.float64)[3]
            + np.asarray(outs[b]["cs"], np.float64)[3]
        )
        d = np.asarray(outs[p]["dv"], np.float64)[3]
        dv[(p, b)] = d
        dv[(b, p)] = d

    nce = 0.0
    for a in range(FLIP):
        for b in range(FLIP):
            if a == b:
                N1 = 2.0 * S10[a]
                Dv = N1 + E10
                half = 10.0 - np.log(Dv) - N1 / Dv
                nce += 2.0 * half.sum()
            else:
                d = dv[(a, b)]
                N1 = S10[a] + S1[(a, b)]
                half = (
                    10.0 * d
                    - np.log(N1)
                    - 1.0
                    - np.log1p(-np.exp(10.0 * d) / N1)
                )
                nce += half.sum()

    # CE: device gives per-row sum(exp(pred)); label logit gathered on host
    se = np.concatenate(
        [np.asarray(outs[c]["m1"], np.float64)[:, 18:22].T.reshape(B) for c in range(FLIP)]
    )
    lab = np.asarray(labels).astype(np.int64)
    pred16 = np.asarray(predicts, np.float32).astype(np.float16)
    xlab = pred16[np.arange(N), lab].astype(np.float64)
    ce = (np.log(se) - xlab).mean()

    val = ALPHA * (-(nce) / 1024.0) + ce
    return np.array(val, dtype=np.float32)


def _run_hw(in_maps, trace=False):
    from concourse.bass_utils import run_bass_kernel_spmd

    nc = _get_nc()
    return run_bass_kernel_spmd(nc, in_maps, core_ids=list(range(FLIP)), trace=trace)


def kernel(predicts, labels, features, indexs=None, **_):
    in_maps = _prep_in_maps(predicts, labels, features)
    res = _run_hw(in_maps)
    return _combine(res.results, predicts, labels)


def kernel_sim(predicts, labels, features, indexs=None, **_):
    """CoreSim (CPU simulator) path for fast correctness iteration."""
    from concourse.bass_interp import CoreSim

    nc = _get_nc()
    in_maps = _prep_in_maps(predicts, labels, features)
    outs = []
    for a in range(FLIP):
        sim = CoreSim(nc, trace=False)
        for k, v in in_maps[a].items():
            sim.tensor(k)[:] = v
        sim.simulate()
        outs.append({k: np.array(sim.tensor(k)) for k in ("m1", "cs", "dv")})
    return _combine(outs, predicts, labels)
